# revision 1
# baseline (speedup 1.0000x reference)
"""TRN2 Bass kernel for nn_KNN_model (conv stack + pairwise patch distances).

Strategy (8 NeuronCores, SPMD):
  - Convs sharded over H: each core computes a 40-row slab (32 owned + 4 halo
    each side) through all 4 conv+BN+ReLU layers in float32r (TF32-like) on PE.
    3x3 conv = 6 matmul streams per tile: 3 K=128 pairs (top+mid tap rows via a
    partition-shifted slab copy) + 3 K=64 singles (bottom tap row).
  - BN stats: per-core partial (mean, var) via bn_stats/bn_aggr over owned rows,
    tiny AllGather + PE ones-matmul reduce, scale/shift transposed to
    per-partition vectors via K=1 matmuls; applied fused in one ACT pass
    (relu(scale*y+shift)) that also casts to f32r for the next conv.
  - Out-of-image halo rows are zeroed with a per-core mask input (SPMD-safe).
  - Final features -> patch matrix (16 x 1024 local) via a strided scatter DMA,
    augmented to 18 rows (p, sq, ones), AllGathered; distance block
    D[i,j] = sqrt(relu(sq_i + sq_j - 2 p_i.p_j)) computed as ONE fp32 K=18
    matmul per [128,512] tile, DVE relu, ACT sqrt, 4MB DMAs out.
"""
import numpy as np
import ml_dtypes
import concourse.bacc as bacc
import concourse.bass as bass
import concourse.tile as tile
from concourse import mybir
from concourse.bass_utils import run_bass_kernel_spmd

F32 = mybir.dt.float32
F32R = mybir.dt.float32r
BF16 = mybir.dt.bfloat16
AF = mybir.ActivationFunctionType
ALU = mybir.AluOpType

NCORES = 8
WP = 258            # padded row width (256 + 2 pad cols)
ROWS = 40           # ext slab rows per core (32 owned + 4 halo each side)
LEAD = 4            # lead margin so tap offsets never go negative
HROWS = 42          # slab rows + 1 pad row top/bottom
HFREE = LEAD + HROWS * WP + 4   # 10844
YFREE = ROWS * WP   # 10320
EPS = 1e-5
GOFF = [0, 64, 128, 192]        # g/be packing offsets per layer
COUT = [64, 64, 64, 2]

_CACHE = {}


def _conv_tiles(s0=0, s1=YFREE):
    out, s = [], s0
    while s < s1:
        L = min(512, s1 - s)
        out.append((s, L))
        s += L
    return out


def build():
    nc = bacc.Bacc(trn_type="TRN2", num_devices=NCORES)
    x0 = nc.dram_tensor("x0", [27, YFREE], F32, kind="ExternalInput").ap()
    w0T = nc.dram_tensor("w0T", [27, 64], F32, kind="ExternalInput").ap()
    wp_in, ws_in = {}, {}
    for l in (1, 2, 3):
        co = COUT[l]
        wp_in[l] = nc.dram_tensor(f"wp{l}", [3, 128, co], F32, kind="ExternalInput").ap()
        ws_in[l] = nc.dram_tensor(f"ws{l}", [3, 64, co], F32, kind="ExternalInput").ap()
    g_all = nc.dram_tensor("g_all", [1, 194], F32, kind="ExternalInput").ap()
    be_all = nc.dram_tensor("be_all", [1, 194], F32, kind="ExternalInput").ap()
    mask8 = nc.dram_tensor("mask8", [1, 8 * WP], F32, kind="ExternalInput").ap()
    out = nc.dram_tensor("out", [1024, 8192], F32, kind="ExternalOutput").ap()

    TILES = {0: _conv_tiles(WP, 39 * WP),
             1: _conv_tiles(2 * WP, 38 * WP),
             2: _conv_tiles(3 * WP, 37 * WP)}

    with tile.TileContext(nc) as tc:
      with tc.tile_pool(name="pers", bufs=1) as pers, \
           tc.tile_pool(name="dr", bufs=1, space="DRAM") as dr:
        gsb = pers.tile([1, 194], F32)
        nc.sync.dma_start(out=gsb, in_=g_all)
        besb = pers.tile([1, 194], F32)
        nc.sync.dma_start(out=besb, in_=be_all)
        ones1 = pers.tile([1, 1], F32)
        nc.vector.memset(ones1, 1.0)
        ones8 = pers.tile([8, 1], F32)
        nc.vector.memset(ones8, 0.125)   # 1/8 for mean-of-cores matmul
        epst = pers.tile([1, 1], F32)
        nc.vector.memset(epst, EPS)

        def bn_finish(l, C, regions, bnps, sbp):
            """Cross-core BN: partial stats -> AllGather -> scale/shift [C,1]."""
            # each region is [C, k, <=512]; bn_stats keeps non-innermost dims
            n = sum(r.shape[1] if r.ndim == 3 else 1 for r in regions)
            st = sbp.tile([C, n, 6], F32, tag=f"st{l}")
            i = 0
            for ap in regions:
                k = ap.shape[1] if ap.ndim == 3 else 1
                o = st[:, i:i + k, :] if ap.ndim == 3 else st[:, i, :]
                nc.vector.bn_stats(out=o, in_=ap)
                i += k
            mvt = sbp.tile([C, 2], F32, tag=f"mv{l}")
            nc.vector.bn_aggr(out=mvt, in_=st)
            sti = dr.tile([C, 2], F32, tag=f"sti{l}")
            sto = dr.tile([NCORES, C, 2], F32, tag=f"sto{l}")
            nc.gpsimd.dma_start(out=sti, in_=mvt)
            nc.gpsimd.collective_compute(
                "AllGather", ALU.bypass,
                replica_groups=[list(range(NCORES))],
                ins=[sti.opt()], outs=[sto.opt()])
            G = sbp.tile([8, 2 * C], F32, tag=f"G{l}")
            nc.sync.dma_start(out=G, in_=sto.rearrange("k c two -> k (c two)"))
            Gv = G.rearrange("p (c two) -> p c two", two=2)
            m2 = sbp.tile([8, C], F32, tag=f"m2{l}")
            nc.vector.tensor_mul(m2, Gv[:, :, 0], Gv[:, :, 0])
            pavg = bnps.tile([1, 2 * C], F32, tag="bn")
            nc.tensor.matmul(pavg, ones8, G, start=True, stop=True)
            pavg2 = bnps.tile([1, C], F32, tag="bn")
            nc.tensor.matmul(pavg2, ones8, m2, start=True, stop=True)
            A1 = sbp.tile([1, 2 * C], F32, tag=f"A1{l}")
            nc.scalar.copy(A1, pavg)
            A2 = sbp.tile([1, C], F32, tag=f"A2{l}")
            nc.scalar.copy(A2, pavg2)
            A1v = A1.rearrange("p (c two) -> p c two", two=2)
            am, av = A1v[:, :, 0], A1v[:, :, 1]
            t1 = sbp.tile([1, C], F32, tag=f"t1{l}")
            nc.vector.tensor_mul(t1, am, am)       # E[m]^2
            t2 = sbp.tile([1, C], F32, tag=f"t2{l}")
            nc.vector.tensor_sub(t2, A2, t1)       # Var(means)
            t3 = sbp.tile([1, C], F32, tag=f"t3{l}")
            nc.vector.tensor_add(t3, t2, av)       # + E[var] = total var
            sd = sbp.tile([1, C], F32, tag=f"sd{l}")
            nc.scalar.activation(sd, t3, AF.Sqrt, bias=epst)
            rs = sbp.tile([1, C], F32, tag=f"rs{l}")
            nc.vector.reciprocal(rs, sd)
            off = GOFF[l]
            scl = sbp.tile([1, C], F32, tag=f"scl{l}")
            nc.vector.tensor_mul(scl, gsb[:, off:off + C], rs)
            sh0 = sbp.tile([1, C], F32, tag=f"sh0{l}")
            nc.vector.tensor_mul(sh0, am, scl)
            sh = sbp.tile([1, C], F32, tag=f"sh{l}")
            nc.vector.tensor_sub(sh, besb[:, off:off + C], sh0)
            psc = bnps.tile([C, 1], F32, tag="bn")
            nc.tensor.matmul(psc, scl, ones1, start=True, stop=True)
            psh = bnps.tile([C, 1], F32, tag="bn")
            nc.tensor.matmul(psh, sh, ones1, start=True, stop=True)
            sbs = sbp.tile([C, 1], F32, tag=f"sbs{l}")
            nc.scalar.copy(sbs, psc)
            sbh = sbp.tile([C, 1], F32, tag=f"sbh{l}")
            nc.scalar.copy(sbh, psh)
            return sbs, sbh

        # ---------------- conv phase ----------------
        with tc.tile_pool(name="cb", bufs=1) as cb, \
             tc.tile_pool(name="hp", bufs=2) as hp, \
             tc.tile_pool(name="cps", bufs=6, space="PSUM") as cps, \
             tc.tile_pool(name="bnps", bufs=2, space="PSUM") as bnps:
            x0t = cb.tile([27, YFREE], F32R)
            nc.gpsimd.dma_start(out=x0t, in_=x0)
            mskf = cb.tile([64, 8 * WP], F32)
            nc.gpsimd.dma_start(out=mskf, in_=mask8.partition_broadcast(64))
            mv_ = mskf.rearrange("p (r c) -> p r c", c=WP)
            w0 = cb.tile([27, 64], F32R)
            nc.gpsimd.dma_start(out=w0, in_=w0T)
            wpair, wsing = {}, {}
            for l in (1, 2, 3):
                co = COUT[l]
                for p in range(3):
                    t = cb.tile([128, co], F32R, tag=f"twp{l}{p}")
                    nc.gpsimd.dma_start(out=t, in_=wp_in[l][p])
                    wpair[(l, p)] = t
                    t2 = cb.tile([64, co], F32R, tag=f"tws{l}{p}")
                    nc.gpsimd.dma_start(out=t2, in_=ws_in[l][p])
                    wsing[(l, p)] = t2

            def finish_layer(l, y):
                """BN + ReLU + mask + build padded f32r slab with shifted copy."""
                yv = y.rearrange("p (r c) -> p r c", c=WP)
                regs = [yv[:, r, 1:257] for r in range(4, 36)]
                sbs, sbh = bn_finish(l, 64, regs, bnps, cb)
                h = hp.tile([128, HFREE], F32R, tag="h")
                nc.scalar.activation(h[0:64, LEAD + WP:LEAD + WP + YFREE], y,
                                     AF.Relu, bias=sbh, scale=sbs)
                hv = h[0:64, LEAD + WP:LEAD + WP + YFREE].rearrange(
                    "p (r c) -> p r c", c=WP)
                nc.vector.tensor_mul(hv[:, 0:4, :], hv[:, 0:4, :], mv_[:, 0:4, :])
                nc.vector.tensor_mul(hv[:, 36:40, :], hv[:, 36:40, :], mv_[:, 4:8, :])
                hcv = h[0:64, LEAD + WP:LEAD + WP + YFREE].rearrange(
                    "p (r c) -> p c r", c=WP)
                nc.vector.memset(hcv[:, 0, :].bitcast(F32), 0.0)
                nc.vector.memset(hcv[:, 257, :].bitcast(F32), 0.0)
                nc.vector.memset(h[0:64, 0:LEAD + WP].bitcast(F32), 0.0)
                nc.vector.memset(h[0:64, LEAD + WP + YFREE:HFREE].bitcast(F32), 0.0)
                nc.vector.tensor_copy(h[64:128, 0:HFREE - WP],
                                      h[0:64, WP:HFREE])
                nc.vector.memset(h[64:128, HFREE - WP:HFREE].bitcast(F32), 0.0)
                return h

            # conv0 (im2col input, K=27, one stream)
            if True:
                y = cb.tile([64, YFREE], F32, tag="y")
                for (s, L) in TILES[0]:
                    ps = cps.tile([64, 512], F32, tag="cps")
                    nc.tensor.matmul(ps[:, 0:L], w0, x0t[:, s:s + L],
                                     start=True, stop=True)
                    nc.scalar.copy(y[:, s:s + L], ps[:, 0:L])
                h = finish_layer(0, y)

            # conv1, conv2 (6 streams: 3 pairs K=128 + 3 singles K=64)
            GROUP = 6
            for l in (1, 2):
                y = cb.tile([64, YFREE], F32, tag="y")
                for g0 in range(0, len(TILES[l]), GROUP):
                    grp = TILES[l][g0:g0 + GROUP]
                    pss = [cps.tile([64, 512], F32, tag="cps", name=f"cps{g0}_{i}")
                           for i in range(len(grp))]
                    for p in range(3):
                        for ps, (s, L) in zip(pss, grp):
                            o = LEAD + 516 + s + p - 1
                            nc.tensor.matmul(ps[:, 0:L], wsing[(l, p)],
                                             h[0:64, o:o + L],
                                             start=(p == 0), stop=False)
                    for p in range(3):
                        for ps, (s, L) in zip(pss, grp):
                            o = LEAD + s + p - 1
                            nc.tensor.matmul(ps[:, 0:L], wpair[(l, p)],
                                             h[0:128, o:o + L],
                                             start=False, stop=(p == 2))
                    for ps, (s, L) in zip(pss, grp):
                        nc.scalar.copy(y[:, s:s + L], ps[:, 0:L])
                h = finish_layer(l, y)

            # conv3: output streamed in patch order (gy, py, px, gx)
            def c3rhs(p0, np_, off):
                wide = h[p0:p0 + np_, off:off + 2 * WP]
                w2 = wide.rearrange("p (py c) -> p py c", py=2)
                w3 = w2[:, :, 0:256]
                return w3.rearrange("p py (gx px) -> p py px gx", px=4)

            y3 = cb.tile([2, 8192], F32, tag="y")
            T3 = [(gy, ph) for gy in range(8) for ph in range(2)]
            for g0 in range(0, 16, 6):
                grp = T3[g0:g0 + 6]
                pss = [cps.tile([64, 512], F32, tag="cps", name=f"cps{g0}_{i}")
                           for i in range(len(grp))]
                bases = [LEAD + (5 + 4 * gy + 2 * ph) * WP + 1 for gy, ph in grp]
                for p in range(3):
                    for ps, base in zip(pss, bases):
                        nc.tensor.matmul(ps[0:2, :], wsing[(3, p)],
                                         c3rhs(0, 64, base + WP + (p - 1)),
                                         start=(p == 0), stop=False)
                for p in range(3):
                    for ps, base in zip(pss, bases):
                        nc.tensor.matmul(ps[0:2, :], wpair[(3, p)],
                                         c3rhs(0, 128, base + (p - 1) - WP),
                                         start=False, stop=(p == 2))
                for ps, (gy, ph) in zip(pss, grp):
                    t = gy * 2 + ph
                    nc.scalar.copy(y3[:, t * 512:(t + 1) * 512], ps[0:2, :])
            regs3 = [y3[:, i * 512:(i + 1) * 512] for i in range(16)]
            sbs3, sbh3 = bn_finish(3, 2, regs3, bnps, cb)
            nc.scalar.activation(y3, y3, AF.Relu, bias=sbh3, scale=sbs3)

            # scatter y3 -> patch-major DRAM [16(k=py*4+px), 1024(c,gy,gx)]
            y3p = dr.tile([16, 1024], F32, tag="y3p")
            y5 = y3.rearrange("p (gy py px gx) -> p gy py px gx",
                              gy=8, py=4, px=4)
            y3pr = y3p.rearrange("k (c gy gx) -> k c gy gx", c=2, gy=8)
            for py in range(4):
                for px in range(4):
                    nc.sync.dma_start(out=y3pr[py * 4 + px],
                                      in_=y5[:, :, py, px, :])

        # ---------------- patch augment + AllGather ----------------
        agin = dr.tile([18, 1024], F32, tag="agin")
        gath = dr.tile([8, 18, 1024], F32, tag="gath")
        with tc.tile_pool(name="db", bufs=1) as db, \
             tc.tile_pool(name="sqps", bufs=2, space="PSUM") as sqps:
            Praw = db.tile([16, 1024], F32)
            nc.sync.dma_start(out=Praw, in_=y3p)
            Q = db.tile([16, 1024], F32)
            nc.vector.tensor_mul(Q, Praw, Praw)
            ones16 = db.tile([16, 1], F32)
            nc.vector.memset(ones16, 1.0)
            sqv = db.tile([1, 1024], F32)
            for j in range(2):
                pq = sqps.tile([1, 512], F32, tag="pq")
                nc.tensor.matmul(pq, ones16, Q[:, j * 512:(j + 1) * 512],
                                 start=True, stop=True)
                nc.scalar.copy(sqv[:, j * 512:(j + 1) * 512], pq)
            B16 = db.tile([16, 1024], F32)
            nc.vector.tensor_scalar_mul(B16, Praw, -2.0)
            ones1k = db.tile([1, 1024], F32)
            nc.vector.memset(ones1k, 1.0)
            nc.sync.dma_start(out=agin[0:16, :], in_=B16)
            nc.sync.dma_start(out=agin[16:17, :], in_=ones1k)
            nc.sync.dma_start(out=agin[17:18, :], in_=sqv)
            nc.gpsimd.collective_compute(
                "AllGather", ALU.bypass,
                replica_groups=[list(range(NCORES))],
                ins=[agin.opt()], outs=[gath.opt()])

        # ---------------- distance phase ----------------
        with tc.tile_pool(name="dist", bufs=1) as dist, \
             tc.tile_pool(name="stg", bufs=2) as stg, \
             tc.tile_pool(name="dps", bufs=8, space="PSUM") as dps:
            lhsT = dist.tile([128, 1024], F32)
            nc.sync.dma_start(out=lhsT[0:16, :], in_=agin[0:16, :])
            nc.sync.dma_start(out=lhsT[16:17, :], in_=agin[17:18, :])
            nc.sync.dma_start(out=lhsT[17:18, :], in_=agin[16:17, :])
            nc.vector.tensor_scalar_mul(lhsT[0:16, :], lhsT[0:16, :], -0.5)
            rhs = dist.tile([128, 8192], F32)
            for j in range(16):
                c, k = j // 8, j % 8
                nc.sync.dma_start(out=rhs[0:18, j * 512:(j + 1) * 512],
                                  in_=gath[k, :, c * 512:(c + 1) * 512])
            # replicate the 18 aug rows into 4 PE row-group strips so 4
            # K=18 matmuls run concurrently (tile_position row packing)
            for b in (32, 64, 96):
                nc.vector.tensor_copy(lhsT[b:b + 18, :], lhsT[0:18, :])
                nc.vector.tensor_copy(rhs[b:b + 18, :], rhs[0:18, :])
            for m in range(8):
                stage = stg.tile([128, 8192], F32, tag="stage")
                for n in range(16):
                    b = 32 * ((m * 16 + n) % 4)
                    ps = dps.tile([128, 512], F32, tag="dp")
                    nc.tensor.matmul(ps, lhsT[b:b + 18, m * 128:(m + 1) * 128],
                                     rhs[b:b + 18, n * 512:(n + 1) * 512],
                                     start=True, stop=True,
                                     tile_position=(b, 0))
                    nc.vector.tensor_scalar_max(stage[:, n * 512:(n + 1) * 512],
                                                ps, 0.0)
                    nc.scalar.activation(stage[:, n * 512:(n + 1) * 512],
                                         stage[:, n * 512:(n + 1) * 512], AF.Sqrt)
                nc.sync.dma_start(out=out[m * 128:(m + 1) * 128, :], in_=stage)
    nc.finalize()
    return nc


def _prep_inputs(x, ws_, gs, bes):
    """Per-core numpy input dicts."""
    BF = ml_dtypes.bfloat16
    xp = np.pad(x[0], ((0, 0), (5, 5), (2, 3))).astype(np.float32)
    w0 = ws_[0]
    w0T = np.ascontiguousarray(
        w0.transpose(2, 3, 1, 0).reshape(27, 64)).astype(np.float32)
    wp, wsg = {}, {}
    for l in (1, 2, 3):
        w = ws_[l]
        wp[l] = np.ascontiguousarray(np.stack(
            [np.concatenate([w[:, :, 0, p].T, w[:, :, 1, p].T], 0)
             for p in range(3)])).astype(np.float32)
        wsg[l] = np.ascontiguousarray(np.stack(
            [w[:, :, 2, p].T for p in range(3)])).astype(np.float32)
    g_all = np.concatenate([np.asarray(g, np.float32).ravel() for g in gs]
                           ).reshape(1, 194)
    be_all = np.concatenate([np.asarray(b, np.float32).ravel() for b in bes]
                            ).reshape(1, 194)
    in_maps = []
    for k in range(NCORES):
        col = np.empty((27, ROWS, WP), np.float32)
        for dy in range(3):
            for dx in range(3):
                for ci in range(3):
                    r0 = 32 * k + dy
                    col[(dy * 3 + dx) * 3 + ci] = xp[ci, r0:r0 + ROWS, dx:dx + WP]
        mask = np.zeros((8, WP), np.float32)
        for i, r in enumerate([0, 1, 2, 3, 36, 37, 38, 39]):
            ir = 32 * k - 4 + r
            if 0 <= ir < 256:
                mask[i, 1:257] = 1.0
        in_maps.append(dict(
            x0=np.ascontiguousarray(col.reshape(27, YFREE)),
            w0T=w0T, wp1=wp[1], ws1=wsg[1], wp2=wp[2], ws2=wsg[2],
            wp3=wp[3], ws3=wsg[3], g_all=g_all, be_all=be_all,
            mask8=np.ascontiguousarray(mask.reshape(1, 8 * WP))))
    return in_maps


def kernel(x, w0, b0, g0, be0, w1, b1, g1, be1, w2, b2, g2, be2,
           w3, b3, g3, be3):
    # conv bias b_i cancels exactly inside BatchNorm (mean absorbs it); unused.
    if "nc" not in _CACHE:
        _CACHE["nc"] = build()
    nc = _CACHE["nc"]
    in_maps = _prep_inputs(
        np.asarray(x, np.float32),
        [np.asarray(w, np.float32) for w in (w0, w1, w2, w3)],
        (g0, g1, g2, g3), (be0, be1, be2, be3))
    res = run_bass_kernel_spmd(nc, in_maps, list(range(NCORES)))
    D = np.empty((8192, 8192), np.float32)
    for k in range(NCORES):
        o = res.results[k]["out"]
        for c in range(2):
            D[c * 4096 + k * 512: c * 4096 + (k + 1) * 512, :] = \
                o[c * 512:(c + 1) * 512, :]
    return D



# revision 7
# speedup vs baseline: 1.0417x; 1.0417x over previous
"""TRN2 Bass kernel for nn_KNN_model (conv stack + pairwise patch distances).

Strategy (8 NeuronCores, SPMD):
  - Convs sharded over H: each core computes a 40-row slab (32 owned + 4 halo
    each side) through all 4 conv+BN+ReLU layers in float32r (TF32-like) on PE.
    3x3 conv = 6 matmul streams per tile: 3 K=128 pairs (top+mid tap rows via a
    partition-shifted slab copy) + 3 K=64 singles (bottom tap row).
  - BN stats: per-core partial (mean, var) via bn_stats/bn_aggr over owned rows,
    tiny AllGather + PE ones-matmul reduce, scale/shift transposed to
    per-partition vectors via K=1 matmuls; applied fused in one ACT pass
    (relu(scale*y+shift)) that also casts to f32r for the next conv.
  - Out-of-image halo rows are zeroed with a per-core mask input (SPMD-safe).
  - Final features -> patch matrix (16 x 1024 local) via a strided scatter DMA,
    augmented to 18 rows (p, sq, ones), AllGathered; distance block
    D[i,j] = sqrt(relu(sq_i + sq_j - 2 p_i.p_j)) computed as ONE fp32 K=18
    matmul per [128,512] tile, DVE relu, ACT sqrt, 4MB DMAs out.
"""
import numpy as np
import ml_dtypes
import concourse.bacc as bacc
import concourse.bass as bass
import concourse.tile as tile
from concourse import mybir
from concourse.bass_utils import run_bass_kernel_spmd

F32 = mybir.dt.float32
F32R = mybir.dt.float32r
BF16 = mybir.dt.bfloat16
AF = mybir.ActivationFunctionType
ALU = mybir.AluOpType

NCORES = 8
WP = 258            # padded row width (256 + 2 pad cols)
ROWS = 40           # ext slab rows per core (32 owned + 4 halo each side)
LEAD = 4            # lead margin so tap offsets never go negative
HROWS = 42          # slab rows + 1 pad row top/bottom
HFREE = LEAD + HROWS * WP + 4   # 10844
YFREE = ROWS * WP   # 10320
EPS = 1e-5
GOFF = [0, 64, 128, 192]        # g/be packing offsets per layer
COUT = [64, 64, 64, 2]

_CACHE = {}


def _conv_tiles(s0=0, s1=YFREE):
    out, s = [], s0
    while s < s1:
        L = min(512, s1 - s)
        out.append((s, L))
        s += L
    return out


def build():
    nc = bacc.Bacc(trn_type="TRN2", num_devices=NCORES)
    x0 = nc.dram_tensor("x0", [27, YFREE], F32, kind="ExternalInput").ap()
    w0T = nc.dram_tensor("w0T", [27, 64], F32, kind="ExternalInput").ap()
    wp_in, ws_in = {}, {}
    for l in (1, 2, 3):
        co = COUT[l]
        wp_in[l] = nc.dram_tensor(f"wp{l}", [3, 128, co], F32, kind="ExternalInput").ap()
        ws_in[l] = nc.dram_tensor(f"ws{l}", [3, 64, co], F32, kind="ExternalInput").ap()
    g_all = nc.dram_tensor("g_all", [1, 194], F32, kind="ExternalInput").ap()
    be_all = nc.dram_tensor("be_all", [1, 194], F32, kind="ExternalInput").ap()
    mask8 = nc.dram_tensor("mask8", [1, 8 * WP], F32, kind="ExternalInput").ap()
    out = nc.dram_tensor("out", [1024, 8192], BF16, kind="ExternalOutput").ap()

    TILES = {0: _conv_tiles(WP, 39 * WP),
             1: _conv_tiles(2 * WP, 38 * WP),
             2: _conv_tiles(3 * WP, 37 * WP)}

    with tile.TileContext(nc) as tc:
      with tc.tile_pool(name="pers", bufs=1) as pers, \
           tc.tile_pool(name="dr", bufs=1, space="DRAM") as dr:
        gsb = pers.tile([1, 194], F32)
        nc.sync.dma_start(out=gsb, in_=g_all)
        besb = pers.tile([1, 194], F32)
        nc.sync.dma_start(out=besb, in_=be_all)
        # tiny warm-up AllGather: absorbs the ~23us first-collective
        # rendezvous/CC-startup latency while conv0 runs
        wu = pers.tile([1, 4], F32)
        nc.vector.memset(wu, 0.0)
        wui = dr.tile([1, 4], F32, tag="wui")
        wuo = dr.tile([NCORES, 4], F32, tag="wuo")
        nc.gpsimd.dma_start(out=wui, in_=wu)
        nc.gpsimd.collective_compute(
            "AllGather", ALU.bypass,
            replica_groups=[list(range(NCORES))],
            ins=[wui.opt()], outs=[wuo.opt()])
        ones1 = pers.tile([1, 1], F32)
        nc.vector.memset(ones1, 1.0)
        ones8 = pers.tile([8, 1], F32)
        nc.vector.memset(ones8, 0.125)   # 1/8 for mean-of-cores matmul
        epst = pers.tile([1, 1], F32)
        nc.vector.memset(epst, EPS)

        def bn_finish(l, C, regions, bnps, sbp):
            """Cross-core BN: partial stats -> AllGather -> scale/shift [C,1]."""
            # each region is [C, k, <=512]; bn_stats keeps non-innermost dims
            n = sum(r.shape[1] if r.ndim == 3 else 1 for r in regions)
            st = sbp.tile([C, n, 6], F32, tag=f"st{l}")
            i = 0
            for ap in regions:
                k = ap.shape[1] if ap.ndim == 3 else 1
                o = st[:, i:i + k, :] if ap.ndim == 3 else st[:, i, :]
                nc.vector.bn_stats(out=o, in_=ap)
                i += k
            mvt = sbp.tile([C, 2], F32, tag=f"mv{l}")
            nc.vector.bn_aggr(out=mvt, in_=st)
            sti = dr.tile([C, 2], F32, tag=f"sti{l}")
            sto = dr.tile([NCORES, C, 2], F32, tag=f"sto{l}")
            nc.gpsimd.dma_start(out=sti, in_=mvt)
            nc.gpsimd.collective_compute(
                "AllGather", ALU.bypass,
                replica_groups=[list(range(NCORES))],
                ins=[sti.opt()], outs=[sto.opt()])
            G = sbp.tile([8, 2 * C], F32, tag=f"G{l}")
            nc.sync.dma_start(out=G, in_=sto.rearrange("k c two -> k (c two)"))
            Gv = G.rearrange("p (c two) -> p c two", two=2)
            m2 = sbp.tile([8, C], F32, tag=f"m2{l}")
            nc.vector.tensor_mul(m2, Gv[:, :, 0], Gv[:, :, 0])
            pavg = bnps.tile([1, 2 * C], F32, tag="bn")
            nc.tensor.matmul(pavg, ones8, G, start=True, stop=True)
            pavg2 = bnps.tile([1, C], F32, tag="bn")
            nc.tensor.matmul(pavg2, ones8, m2, start=True, stop=True)
            A1 = sbp.tile([1, 2 * C], F32, tag=f"A1{l}")
            nc.scalar.copy(A1, pavg)
            A2 = sbp.tile([1, C], F32, tag=f"A2{l}")
            nc.scalar.copy(A2, pavg2)
            A1v = A1.rearrange("p (c two) -> p c two", two=2)
            am, av = A1v[:, :, 0], A1v[:, :, 1]
            t1 = sbp.tile([1, C], F32, tag=f"t1{l}")
            nc.vector.tensor_mul(t1, am, am)       # E[m]^2
            t2 = sbp.tile([1, C], F32, tag=f"t2{l}")
            nc.vector.tensor_sub(t2, A2, t1)       # Var(means)
            t3 = sbp.tile([1, C], F32, tag=f"t3{l}")
            nc.vector.tensor_add(t3, t2, av)       # + E[var] = total var
            sd = sbp.tile([1, C], F32, tag=f"sd{l}")
            nc.scalar.activation(sd, t3, AF.Sqrt, bias=epst)
            rs = sbp.tile([1, C], F32, tag=f"rs{l}")
            nc.vector.reciprocal(rs, sd)
            off = GOFF[l]
            scl = sbp.tile([1, C], F32, tag=f"scl{l}")
            nc.vector.tensor_mul(scl, gsb[:, off:off + C], rs)
            sh0 = sbp.tile([1, C], F32, tag=f"sh0{l}")
            nc.vector.tensor_mul(sh0, am, scl)
            sh = sbp.tile([1, C], F32, tag=f"sh{l}")
            nc.vector.tensor_sub(sh, besb[:, off:off + C], sh0)
            psc = bnps.tile([C, 1], F32, tag="bn")
            nc.tensor.matmul(psc, scl, ones1, start=True, stop=True)
            psh = bnps.tile([C, 1], F32, tag="bn")
            nc.tensor.matmul(psh, sh, ones1, start=True, stop=True)
            sbs = sbp.tile([C, 1], F32, tag=f"sbs{l}")
            nc.scalar.copy(sbs, psc)
            sbh = sbp.tile([C, 1], F32, tag=f"sbh{l}")
            nc.scalar.copy(sbh, psh)
            return sbs, sbh

        # ---------------- conv phase ----------------
        with tc.tile_pool(name="cb", bufs=1) as cb, \
             tc.tile_pool(name="hp", bufs=2) as hp, \
             tc.tile_pool(name="cps", bufs=6, space="PSUM") as cps, \
             tc.tile_pool(name="bnps", bufs=2, space="PSUM") as bnps:
            x0t = cb.tile([27, YFREE], F32R)
            nc.gpsimd.dma_start(out=x0t, in_=x0)
            mskf = cb.tile([64, 8 * WP], F32)
            nc.gpsimd.dma_start(out=mskf, in_=mask8.partition_broadcast(64))
            mv_ = mskf.rearrange("p (r c) -> p r c", c=WP)
            w0 = cb.tile([27, 64], F32R)
            nc.gpsimd.dma_start(out=w0, in_=w0T)
            wpair, wsing = {}, {}
            for l in (1, 2, 3):
                co = COUT[l]
                for p in range(3):
                    t = cb.tile([128, co], F32R, tag=f"twp{l}{p}")
                    nc.gpsimd.dma_start(out=t, in_=wp_in[l][p])
                    wpair[(l, p)] = t
                    t2 = cb.tile([64, co], F32R, tag=f"tws{l}{p}")
                    nc.gpsimd.dma_start(out=t2, in_=ws_in[l][p])
                    wsing[(l, p)] = t2

            def finish_layer(l, y):
                """BN + ReLU + mask + build padded f32r slab with shifted copy."""
                yv = y.rearrange("p (r c) -> p r c", c=WP)
                regs = [yv[:, r, 1:257] for r in range(4, 36)]
                sbs, sbh = bn_finish(l, 64, regs, bnps, cb)
                h = hp.tile([128, HFREE], F32R, tag="h")
                nc.scalar.activation(h[0:64, LEAD + WP:LEAD + WP + YFREE], y,
                                     AF.Relu, bias=sbh, scale=sbs)
                hv = h[0:64, LEAD + WP:LEAD + WP + YFREE].rearrange(
                    "p (r c) -> p r c", c=WP)
                nc.vector.tensor_mul(hv[:, 0:4, :], hv[:, 0:4, :], mv_[:, 0:4, :])
                nc.vector.tensor_mul(hv[:, 36:40, :], hv[:, 36:40, :], mv_[:, 4:8, :])
                hcv = h[0:64, LEAD + WP:LEAD + WP + YFREE].rearrange(
                    "p (r c) -> p c r", c=WP)
                nc.vector.memset(hcv[:, 0, :].bitcast(F32), 0.0)
                nc.vector.memset(hcv[:, 257, :].bitcast(F32), 0.0)
                nc.vector.memset(h[0:64, 0:LEAD + WP].bitcast(F32), 0.0)
                nc.vector.memset(h[0:64, LEAD + WP + YFREE:HFREE].bitcast(F32), 0.0)
                nc.vector.tensor_copy(h[64:128, 0:HFREE - WP],
                                      h[0:64, WP:HFREE])
                nc.vector.memset(h[64:128, HFREE - WP:HFREE].bitcast(F32), 0.0)
                return h

            # conv0 (im2col input, K=27, one stream)
            if True:
                y = cb.tile([64, YFREE], F32, tag="y")
                for (s, L) in TILES[0]:
                    ps = cps.tile([64, 512], F32, tag="cps")
                    nc.tensor.matmul(ps[:, 0:L], w0, x0t[:, s:s + L],
                                     start=True, stop=True)
                    nc.scalar.copy(y[:, s:s + L], ps[:, 0:L])
                h = finish_layer(0, y)

            # conv1, conv2 (6 streams: 3 pairs K=128 + 3 singles K=64)
            GROUP = 6
            for l in (1, 2):
                y = cb.tile([64, YFREE], F32, tag="y")
                for g0 in range(0, len(TILES[l]), GROUP):
                    grp = TILES[l][g0:g0 + GROUP]
                    pss = [cps.tile([64, 512], F32, tag="cps", name=f"cps{g0}_{i}")
                           for i in range(len(grp))]
                    for p in range(3):
                        for ps, (s, L) in zip(pss, grp):
                            o = LEAD + 516 + s + p - 1
                            nc.tensor.matmul(ps[:, 0:L], wsing[(l, p)],
                                             h[0:64, o:o + L],
                                             start=(p == 0), stop=False)
                    for p in range(3):
                        for ps, (s, L) in zip(pss, grp):
                            o = LEAD + s + p - 1
                            nc.tensor.matmul(ps[:, 0:L], wpair[(l, p)],
                                             h[0:128, o:o + L],
                                             start=False, stop=(p == 2))
                    for ps, (s, L) in zip(pss, grp):
                        nc.scalar.copy(y[:, s:s + L], ps[:, 0:L])
                h = finish_layer(l, y)

            # conv3: output streamed in patch order (gy, py, px, gx)
            def c3rhs(p0, np_, off):
                wide = h[p0:p0 + np_, off:off + 2 * WP]
                w2 = wide.rearrange("p (py c) -> p py c", py=2)
                w3 = w2[:, :, 0:256]
                return w3.rearrange("p py (gx px) -> p py px gx", px=4)

            y3 = cb.tile([2, 8192], F32, tag="y")
            T3 = [(gy, ph) for gy in range(8) for ph in range(2)]
            for g0 in range(0, 16, 6):
                grp = T3[g0:g0 + 6]
                pss = [cps.tile([64, 512], F32, tag="cps", name=f"cps{g0}_{i}")
                           for i in range(len(grp))]
                bases = [LEAD + (5 + 4 * gy + 2 * ph) * WP + 1 for gy, ph in grp]
                for p in range(3):
                    for ps, base in zip(pss, bases):
                        nc.tensor.matmul(ps[0:2, :], wsing[(3, p)],
                                         c3rhs(0, 64, base + WP + (p - 1)),
                                         start=(p == 0), stop=False)
                for p in range(3):
                    for ps, base in zip(pss, bases):
                        nc.tensor.matmul(ps[0:2, :], wpair[(3, p)],
                                         c3rhs(0, 128, base + (p - 1) - WP),
                                         start=False, stop=(p == 2))
                for ps, (gy, ph) in zip(pss, grp):
                    t = gy * 2 + ph
                    nc.scalar.copy(y3[:, t * 512:(t + 1) * 512], ps[0:2, :])
            regs3 = [y3[:, i * 512:(i + 1) * 512] for i in range(16)]
            sbs3, sbh3 = bn_finish(3, 2, regs3, bnps, cb)
            nc.scalar.activation(y3, y3, AF.Relu, bias=sbh3, scale=sbs3)

            # scatter y3 -> patch-major DRAM [16(k=py*4+px), 1024(c,gy,gx)]
            y3p = dr.tile([16, 1024], F32, tag="y3p")
            y5 = y3.rearrange("p (gy py px gx) -> p gy py px gx",
                              gy=8, py=4, px=4)
            y3pr = y3p.rearrange("k (c gy gx) -> k c gy gx", c=2, gy=8)
            for py in range(4):
                for px in range(4):
                    nc.sync.dma_start(out=y3pr[py * 4 + px],
                                      in_=y5[:, :, py, px, :])

        # ---------------- patch augment + AllGather ----------------
        agin = dr.tile([18, 1024], F32, tag="agin")
        gath = dr.tile([8, 18, 1024], F32, tag="gath")
        with tc.tile_pool(name="db", bufs=1) as db, \
             tc.tile_pool(name="sqps", bufs=2, space="PSUM") as sqps:
            Praw = db.tile([16, 1024], F32)
            nc.sync.dma_start(out=Praw, in_=y3p)
            Q = db.tile([16, 1024], F32)
            nc.vector.tensor_mul(Q, Praw, Praw)
            ones16 = db.tile([16, 1], F32)
            nc.vector.memset(ones16, 1.0)
            sqv = db.tile([1, 1024], F32)
            for j in range(2):
                pq = sqps.tile([1, 512], F32, tag="pq")
                nc.tensor.matmul(pq, ones16, Q[:, j * 512:(j + 1) * 512],
                                 start=True, stop=True)
                nc.scalar.copy(sqv[:, j * 512:(j + 1) * 512], pq)
            B16 = db.tile([16, 1024], F32)
            nc.vector.tensor_scalar_mul(B16, Praw, -2.0)
            ones1k = db.tile([1, 1024], F32)
            nc.vector.memset(ones1k, 1.0)
            nc.sync.dma_start(out=agin[0:16, :], in_=B16)
            nc.sync.dma_start(out=agin[16:17, :], in_=ones1k)
            nc.sync.dma_start(out=agin[17:18, :], in_=sqv)
            nc.gpsimd.collective_compute(
                "AllGather", ALU.bypass,
                replica_groups=[list(range(NCORES))],
                ins=[agin.opt()], outs=[gath.opt()])

        # ---------------- distance phase ----------------
        # Symmetric-triangle scheme: c0 rows (m 0-3) compute all 16 col
        # blocks; c1 rows (m 4-7) compute only the c1 half (n 8-15). The
        # host mirrors the lower-left cross quadrant from the upper-right.
        # Matmuls in f32r (1 cyc/row vs 4 for fp32); relu alternates
        # vector/gpsimd; sqrt on scalar casts to bf16 for half the out DMA.
        with tc.tile_pool(name="dist", bufs=1) as dist, \
             tc.tile_pool(name="stg", bufs=2) as stg, \
             tc.tile_pool(name="dps", bufs=8, space="PSUM") as dps:
            lhsT = dist.tile([128, 1024], F32R)
            nc.gpsimd.dma_start(out=lhsT[0:16, :], in_=agin[0:16, :])
            nc.gpsimd.dma_start(out=lhsT[16:17, :], in_=agin[17:18, :])
            nc.gpsimd.dma_start(out=lhsT[17:18, :], in_=agin[16:17, :])
            nc.vector.tensor_scalar_mul(lhsT[0:16, :], lhsT[0:16, :], -0.5)
            rhs = dist.tile([128, 8192], F32R)
            for j in range(16):
                c, k = j // 8, j % 8
                nc.gpsimd.dma_start(out=rhs[0:18, j * 512:(j + 1) * 512],
                                    in_=gath[k, :, c * 512:(c + 1) * 512])
            # replicate the 18 aug rows into 4 PE row-group strips so 4
            # K=18 matmuls run concurrently (tile_position row packing)
            for b in (32, 64, 96):
                nc.vector.tensor_copy(lhsT[b:b + 18, :], lhsT[0:18, :])
                nc.vector.tensor_copy(rhs[b:b + 18, :], rhs[0:18, :])
            ti = 0
            for m in range(8):
                nlist = range(16) if m < 4 else range(8, 16)
                stage = stg.tile([128, 8192], BF16, tag="stage")
                for n in nlist:
                    b = 32 * (ti % 4)
                    sl = stage[:, n * 512:(n + 1) * 512]
                    ps = dps.tile([128, 512], F32, tag="dp")
                    nc.tensor.matmul(ps, lhsT[b:b + 18, m * 128:(m + 1) * 128],
                                     rhs[b:b + 18, n * 512:(n + 1) * 512],
                                     start=True, stop=True,
                                     tile_position=(b, 0))
                    if ti % 4 == 3:   # balance: every 4th relu on scalar
                        nc.scalar.activation(sl, ps, AF.Relu)
                    else:
                        nc.vector.tensor_scalar_max(sl, ps, 0.0)
                    nc.scalar.activation(sl, sl, AF.Sqrt)
                    ti += 1
                c0 = 0 if m < 4 else 4096
                nc.sync.dma_start(out=out[m * 128:(m + 1) * 128, c0:8192],
                                  in_=stage[:, c0:8192])
    nc.finalize()
    return nc


def _prep_inputs(x, ws_, gs, bes):
    """Per-core numpy input dicts."""
    BF = ml_dtypes.bfloat16
    xp = np.pad(x[0], ((0, 0), (5, 5), (2, 3))).astype(np.float32)
    w0 = ws_[0]
    w0T = np.ascontiguousarray(
        w0.transpose(2, 3, 1, 0).reshape(27, 64)).astype(np.float32)
    wp, wsg = {}, {}
    for l in (1, 2, 3):
        w = ws_[l]
        wp[l] = np.ascontiguousarray(np.stack(
            [np.concatenate([w[:, :, 0, p].T, w[:, :, 1, p].T], 0)
             for p in range(3)])).astype(np.float32)
        wsg[l] = np.ascontiguousarray(np.stack(
            [w[:, :, 2, p].T for p in range(3)])).astype(np.float32)
    g_all = np.concatenate([np.asarray(g, np.float32).ravel() for g in gs]
                           ).reshape(1, 194)
    be_all = np.concatenate([np.asarray(b, np.float32).ravel() for b in bes]
                            ).reshape(1, 194)
    in_maps = []
    for k in range(NCORES):
        col = np.empty((27, ROWS, WP), np.float32)
        for dy in range(3):
            for dx in range(3):
                for ci in range(3):
                    r0 = 32 * k + dy
                    col[(dy * 3 + dx) * 3 + ci] = xp[ci, r0:r0 + ROWS, dx:dx + WP]
        mask = np.zeros((8, WP), np.float32)
        for i, r in enumerate([0, 1, 2, 3, 36, 37, 38, 39]):
            ir = 32 * k - 4 + r
            if 0 <= ir < 256:
                mask[i, 1:257] = 1.0
        in_maps.append(dict(
            x0=np.ascontiguousarray(col.reshape(27, YFREE)),
            w0T=w0T, wp1=wp[1], ws1=wsg[1], wp2=wp[2], ws2=wsg[2],
            wp3=wp[3], ws3=wsg[3], g_all=g_all, be_all=be_all,
            mask8=np.ascontiguousarray(mask.reshape(1, 8 * WP))))
    return in_maps


def kernel(x, w0, b0, g0, be0, w1, b1, g1, be1, w2, b2, g2, be2,
           w3, b3, g3, be3):
    # conv bias b_i cancels exactly inside BatchNorm (mean absorbs it); unused.
    if "nc" not in _CACHE:
        _CACHE["nc"] = build()
    nc = _CACHE["nc"]
    in_maps = _prep_inputs(
        np.asarray(x, np.float32),
        [np.asarray(w, np.float32) for w in (w0, w1, w2, w3)],
        (g0, g1, g2, g3), (be0, be1, be2, be3))
    res = run_bass_kernel_spmd(nc, in_maps, list(range(NCORES)))
    D = np.empty((8192, 8192), np.float32)
    for k in range(NCORES):
        o = res.results[k]["out"]  # [1024, 8192] bf16
        D[k * 512:(k + 1) * 512, :] = o[0:512, :].astype(np.float32)
        D[4096 + k * 512: 4096 + (k + 1) * 512, 4096:] = \
            o[512:1024, 4096:].astype(np.float32)
    D[4096:, :4096] = D[:4096, 4096:].T
    return D



# revision 24
# speedup vs baseline: 1.1146x; 1.0700x over previous
"""TRN2 Bass kernel for nn_KNN_model (conv stack + pairwise patch distances).

Strategy (8 NeuronCores, SPMD):
  - Convs sharded over H: each core computes a 40-row slab (32 owned + 4 halo
    each side) through all 4 conv+BN+ReLU layers in float32r (TF32-like) on PE.
    3x3 conv = 6 matmul streams per tile: 3 K=128 pairs (top+mid tap rows via a
    partition-shifted slab copy) + 3 K=64 singles (bottom tap row).
  - conv3 (2 out ch): wide-stationary form - one K=128 stream (top+mid rows,
    3 tap-cols x 2 ch = 6 psum rows) + one K=64 stream (bottom row) into the
    same [6,512] psum; DVE shift-adds combine the 3 tap columns.
  - BN stats: per-core partial (mean, var) via one 3D bn_stats/bn_aggr over
    owned rows, tiny AllGather + PE ones-matmul reduce; layers 0-2 transpose
    scale/shift to [C,1] via K=1 matmuls and apply fused in one ACT pass;
    layer 3 applies BN post-scatter on the [16,1024] patch tile.
  - Out-of-image halo rows are zeroed with a per-core mask input (SPMD-safe).
  - Patches [16,1024] + (ones, sq) rows AllGathered; D tile = one f32r K=18
    matmul per [128,512] (4-way tile_position row packing), DVE relu (bf16),
    ACT sqrt (bf16). Symmetric-triangle: c0 rows compute all 16 col blocks,
    c1 rows only the c1 half; host mirrors the lower-left quadrant, diag=0.
"""
import numpy as np
import ml_dtypes
import concourse.bacc as bacc
import concourse.bass as bass
import concourse.tile as tile
from concourse import mybir
from concourse.bass_utils import run_bass_kernel_spmd

F32 = mybir.dt.float32
F32R = mybir.dt.float32r
BF16 = mybir.dt.bfloat16
AF = mybir.ActivationFunctionType
ALU = mybir.AluOpType

NCORES = 8
WP = 258            # padded row width (256 + 2 pad cols)
ROWS = 40           # ext slab rows per core (32 owned + 4 halo each side)
LEAD = 4            # lead margin so tap offsets never go negative
HROWS = 42          # slab rows + 1 pad row top/bottom
HFREE = LEAD + HROWS * WP + 4   # 10844
YFREE = ROWS * WP   # 10320
EPS = 1e-5
GOFF = [0, 64, 128, 192]        # g/be packing offsets per layer
COUT = [64, 64, 64, 2]

_CACHE = {}


def _conv_tiles(s0=0, s1=YFREE):
    out, s = [], s0
    while s < s1:
        L = min(512, s1 - s)
        out.append((s, L))
        s += L
    return out


def _c3_tiles():
    # conv3 tiles: moving [q0, q0+n), outputs [q0+1, q0+n-1)
    q_lo, q_hi = 4 * WP, 36 * WP
    out, q0 = [], q_lo - 1
    while q0 + 1 < q_hi:
        n = min(512, q_hi - q0 + 1)
        out.append((q0, n))
        q0 += 510
    return out


def build():
    nc = bacc.Bacc(trn_type="TRN2", num_devices=NCORES)
    x0 = nc.dram_tensor("x0", [27, YFREE], F32, kind="ExternalInput").ap()
    w0T = nc.dram_tensor("w0T", [27, 64], F32, kind="ExternalInput").ap()
    wp_in, ws_in = {}, {}
    for l in (1, 2):
        wp_in[l] = nc.dram_tensor(f"wp{l}", [3, 128, 64], F32, kind="ExternalInput").ap()
        ws_in[l] = nc.dram_tensor(f"ws{l}", [3, 64, 64], F32, kind="ExternalInput").ap()
    wA3_in = nc.dram_tensor("wA3", [128, 66], F32, kind="ExternalInput").ap()
    wB3_in = nc.dram_tensor("wB3", [64, 66], F32, kind="ExternalInput").ap()
    g_all = nc.dram_tensor("g_all", [1, 194], F32, kind="ExternalInput").ap()
    be_all = nc.dram_tensor("be_all", [1, 194], F32, kind="ExternalInput").ap()
    mask8 = nc.dram_tensor("mask8", [1, 8 * WP], F32, kind="ExternalInput").ap()
    out = nc.dram_tensor("out", [1024, 8192], BF16, kind="ExternalOutput").ap()

    TILES = {0: _conv_tiles(WP, 39 * WP),
             1: _conv_tiles(2 * WP, 38 * WP),
             2: _conv_tiles(3 * WP, 37 * WP)}

    with tile.TileContext(nc) as tc:
      with tc.tile_pool(name="pers", bufs=1) as pers, \
           tc.tile_pool(name="dr", bufs=1, space="DRAM") as dr:
        gsb = pers.tile([1, 194], F32)
        nc.sync.dma_start(out=gsb, in_=g_all)
        besb = pers.tile([1, 194], F32)
        nc.sync.dma_start(out=besb, in_=be_all)
        ones1 = pers.tile([1, 1], F32)
        nc.vector.memset(ones1, 1.0)
        ones8 = pers.tile([8, 1], F32)
        nc.vector.memset(ones8, 0.125)   # 1/8 for mean-of-cores matmul
        epst = pers.tile([1, 1], F32)
        nc.vector.memset(epst, EPS)

        def bn_finish(l, C, regions, bnps, sbp, transpose=True):
            """Cross-core BN: partial stats -> AllGather -> scale/shift.

            Returns ([C,1] scale, [C,1] shift) when transpose else two
            [1,C] row vectors."""
            n = sum(r.shape[1] if r.ndim == 3 else 1 for r in regions)
            st = sbp.tile([C, n, 6], F32, tag=f"st{l}")
            i = 0
            for ap in regions:
                k = ap.shape[1] if ap.ndim == 3 else 1
                o = st[:, i:i + k, :] if ap.ndim == 3 else st[:, i, :]
                nc.vector.bn_stats(out=o, in_=ap)
                i += k
            mvt = sbp.tile([C, 2], F32, tag=f"mv{l}")
            nc.vector.bn_aggr(out=mvt, in_=st)
            sti = dr.tile([C, 2], F32, tag=f"sti{l}")
            sto = dr.tile([NCORES, C, 2], F32, tag=f"sto{l}")
            nc.gpsimd.dma_start(out=sti, in_=mvt)
            nc.gpsimd.collective_compute(
                "AllGather", ALU.bypass,
                replica_groups=[list(range(NCORES))],
                ins=[sti.opt()], outs=[sto.opt()])
            G = sbp.tile([8, 2 * C], F32, tag=f"G{l}")
            nc.sync.dma_start(out=G, in_=sto.rearrange("k c two -> k (c two)"))
            Gv = G.rearrange("p (c two) -> p c two", two=2)
            m2 = sbp.tile([8, C], F32, tag=f"m2{l}")
            nc.vector.tensor_mul(m2, Gv[:, :, 0], Gv[:, :, 0])
            pavg = bnps.tile([1, 2 * C], F32, tag="bn")
            nc.tensor.matmul(pavg, ones8, G, start=True, stop=True)
            pavg2 = bnps.tile([1, C], F32, tag="bn")
            nc.tensor.matmul(pavg2, ones8, m2, start=True, stop=True)
            A1 = sbp.tile([1, 2 * C], F32, tag=f"A1{l}")
            nc.scalar.copy(A1, pavg)
            A2 = sbp.tile([1, C], F32, tag=f"A2{l}")
            nc.scalar.copy(A2, pavg2)
            A1v = A1.rearrange("p (c two) -> p c two", two=2)
            am, av = A1v[:, :, 0], A1v[:, :, 1]
            t1 = sbp.tile([1, C], F32, tag=f"t1{l}")
            nc.vector.tensor_mul(t1, am, am)       # E[m]^2
            t2 = sbp.tile([1, C], F32, tag=f"t2{l}")
            nc.vector.tensor_sub(t2, A2, t1)       # Var(means)
            t3 = sbp.tile([1, C], F32, tag=f"t3{l}")
            nc.vector.tensor_add(t3, t2, av)       # + E[var] = total var
            sd = sbp.tile([1, C], F32, tag=f"sd{l}")
            nc.scalar.activation(sd, t3, AF.Sqrt, bias=epst)
            rs = sbp.tile([1, C], F32, tag=f"rs{l}")
            nc.vector.reciprocal(rs, sd)
            off = GOFF[l]
            scl = sbp.tile([1, C], F32, tag=f"scl{l}")
            nc.vector.tensor_mul(scl, gsb[:, off:off + C], rs)
            sh0 = sbp.tile([1, C], F32, tag=f"sh0{l}")
            nc.vector.tensor_mul(sh0, am, scl)
            sh = sbp.tile([1, C], F32, tag=f"sh{l}")
            nc.vector.tensor_sub(sh, besb[:, off:off + C], sh0)
            if not transpose:
                return scl, sh
            psc = bnps.tile([C, 1], F32, tag="bn")
            nc.tensor.matmul(psc, scl, ones1, start=True, stop=True)
            psh = bnps.tile([C, 1], F32, tag="bn")
            nc.tensor.matmul(psh, sh, ones1, start=True, stop=True)
            sbs = sbp.tile([C, 1], F32, tag=f"sbs{l}")
            nc.scalar.copy(sbs, psc)
            sbh = sbp.tile([C, 1], F32, tag=f"sbh{l}")
            nc.scalar.copy(sbh, psh)
            return sbs, sbh

        # ---------------- conv phase ----------------
        with tc.tile_pool(name="cb", bufs=1) as cb, \
             tc.tile_pool(name="hp", bufs=1) as hp, \
             tc.tile_pool(name="c3p", bufs=2) as c3p, \
             tc.tile_pool(name="cps", bufs=6, space="PSUM") as cps, \
             tc.tile_pool(name="bnps", bufs=2, space="PSUM") as bnps:
            x0t = cb.tile([27, YFREE], F32R)
            nc.gpsimd.dma_start(out=x0t, in_=x0)
            w0 = cb.tile([27, 64], F32R)
            nc.gpsimd.dma_start(out=w0, in_=w0T)
            wpair, wsing = {}, {}
            for l in (1, 2):
                wpl = cb.tile([128, 192], F32R, tag=f"twp{l}")
                nc.gpsimd.dma_start(
                    out=wpl.rearrange("p (t c) -> p t c", t=3),
                    in_=wp_in[l].rearrange("t p c -> p t c"))
                wsl = cb.tile([64, 192], F32R, tag=f"tws{l}")
                nc.gpsimd.dma_start(
                    out=wsl.rearrange("p (t c) -> p t c", t=3),
                    in_=ws_in[l].rearrange("t p c -> p t c"))
                for p in range(3):
                    wpair[(l, p)] = wpl[:, p * 64:(p + 1) * 64]
                    wsing[(l, p)] = wsl[:, p * 64:(p + 1) * 64]
            wA3 = cb.tile([128, 66], F32R)
            nc.gpsimd.dma_start(out=wA3, in_=wA3_in)
            wB3 = cb.tile([64, 66], F32R)
            nc.gpsimd.dma_start(out=wB3, in_=wB3_in)
            mskf = cb.tile([64, 8 * WP], F32)
            nc.sync.dma_start(out=mskf, in_=mask8.partition_broadcast(64))
            mv_ = mskf.rearrange("p (r c) -> p r c", c=WP)

            def finish_layer(l, y):
                """BN + ReLU + mask + build padded f32r slab with shifted copy."""
                yv = y.rearrange("p (r c) -> p r c", c=WP)
                regs = [yv[:, r, 1:257] for r in range(4, 36)]
                sbs, sbh = bn_finish(l, 64, regs, bnps, cb)
                h = hp.tile([128, HFREE], F32R, tag="h")
                nc.scalar.activation(h[0:64, LEAD + WP:LEAD + WP + YFREE], y,
                                     AF.Relu, bias=sbh, scale=sbs)
                hv = h[0:64, LEAD + WP:LEAD + WP + YFREE].rearrange(
                    "p (r c) -> p r c", c=WP)
                nc.vector.tensor_mul(hv[:, 0:4, :], hv[:, 0:4, :], mv_[:, 0:4, :])
                nc.vector.tensor_mul(hv[:, 36:40, :], hv[:, 36:40, :], mv_[:, 4:8, :])
                hcv = h[0:64, LEAD + WP:LEAD + WP + YFREE].rearrange(
                    "p (r c) -> p c r", c=WP)
                nc.vector.memset(hcv[:, 0, :].bitcast(F32), 0.0)
                nc.vector.memset(hcv[:, 257, :].bitcast(F32), 0.0)
                nc.vector.memset(h[0:64, 0:LEAD + WP].bitcast(F32), 0.0)
                nc.vector.memset(h[0:64, LEAD + WP + YFREE:HFREE].bitcast(F32), 0.0)
                nc.vector.tensor_copy(h[64:128, 0:HFREE - WP],
                                      h[0:64, WP:HFREE])
                nc.vector.memset(h[64:128, HFREE - WP:HFREE].bitcast(F32), 0.0)
                return h

            # conv0 (im2col input, K=27, one stream)
            if True:
                y = cb.tile([64, YFREE], F32, tag="y")
                for (s, L) in TILES[0]:
                    ps = cps.tile([64, 512], F32, tag="cps")
                    nc.tensor.matmul(ps[:, 0:L], w0, x0t[:, s:s + L],
                                     start=True, stop=True)
                    nc.scalar.copy(y[:, s:s + L], ps[:, 0:L])
                h = finish_layer(0, y)

            # conv1, conv2 (6 streams: 3 pairs K=128 + 3 singles K=64)
            GROUP = 6
            for l in (1, 2):
                y = cb.tile([64, YFREE], F32, tag="y")
                for g0 in range(0, len(TILES[l]), GROUP):
                    grp = TILES[l][g0:g0 + GROUP]
                    pss = [cps.tile([64, 512], F32, tag="cps", name=f"cps{g0}_{i}")
                           for i in range(len(grp))]
                    for p in range(3):
                        for ps, (s, L) in zip(pss, grp):
                            o = LEAD + 516 + s + p - 1
                            nc.tensor.matmul(ps[:, 0:L],
                                             wsing[(l, p)],
                                             h[0:64, o:o + L],
                                             start=(p == 0), stop=False)
                    for p in range(3):
                        for ps, (s, L) in zip(pss, grp):
                            o = LEAD + s + p - 1
                            nc.tensor.matmul(ps[:, 0:L],
                                             wpair[(l, p)],
                                             h[0:128, o:o + L],
                                             start=False, stop=(p == 2))
                    for ps, (s, L) in zip(pss, grp):
                        nc.scalar.copy(y[:, s:s + L], ps[:, 0:L])
                h = finish_layer(l, y)

            # conv3: wide-stationary, patch-order moving. Stream A (K=128):
            # rows (r-1, r), 6 stationary cols (3 tap-cols x 2 ch); stream B
            # (K=64): row r+1, accumulated into the same [6,512] psum. DVE
            # slice-adds combine the 3 tap columns (px/gx shifts, edges via
            # slice bounds = implicit zero pad). Output lands patch-major.
            def c3mov(p0, np_, off):
                wide = h[p0:p0 + np_, off:off + 2 * WP]
                w2 = wide.rearrange("p (py c) -> p py c", py=2)[:, :, 0:256]
                return w2.rearrange("p py (gx px) -> p py px gx", px=4)

            y3rf = cb.tile([64, YFREE], F32, tag="y")
            y3 = y3rf[0:2, 0:8192]
            for gy in range(8):
                for ph in range(2):
                    offA = LEAD + (4 + 4 * gy + 2 * ph) * WP + 1
                    ps3 = cps.tile([66, 512], F32, tag="cps",
                                   name=f"c3_{gy}_{ph}")
                    nc.tensor.matmul(ps3, wA3,
                                     c3mov(0, 128, offA),
                                     start=True, stop=False)
                    nc.tensor.matmul(ps3, wB3,
                                     c3mov(0, 64, offA + 2 * WP),
                                     start=False, stop=True)
                    t0 = (gy * 2 + ph) * 512
                    yt = y3[:, t0:t0 + 512].rearrange(
                        "p (py px gx) -> p py px gx", py=2, px=4)
                    sb3 = c3p.tile([2, 1536], F32, tag="c3sb",
                                   name=f"c3sb{gy}_{ph}")
                    for g in range(3):
                        nc.scalar.copy(sb3[:, g * 512:(g + 1) * 512],
                                       ps3[32 * g:32 * g + 2, :])
                    pv = sb3.rearrange("p (t py px gx) -> p t py px gx",
                                       t=3, py=2, px=4)
                    nc.vector.tensor_copy(yt[:, :, 0, :], pv[:, 1, :, 0, :])
                    nc.vector.tensor_add(yt[:, :, 1:4, :], pv[:, 1, :, 1:4, :],
                                         pv[:, 0, :, 0:3, :])
                    nc.vector.tensor_add(yt[:, :, 0, 1:], yt[:, :, 0, 1:],
                                         pv[:, 0, :, 3, 0:63])
                    nc.vector.tensor_add(yt[:, :, 0:3, :], yt[:, :, 0:3, :],
                                         pv[:, 2, :, 1:4, :])
                    nc.vector.tensor_add(yt[:, :, 3, 0:63], yt[:, :, 3, 0:63],
                                         pv[:, 2, :, 0, 1:64])
            scl3, sh3 = bn_finish(3, 2,
                                  [y3[:, i * 512:(i + 1) * 512]
                                   for i in range(16)], bnps, cb,
                                  transpose=False)

            # ---------------- patch build + AllGather ----------------
            # scatter raw conv3 rows -> patch-major [16,1024], then apply BN3
            # (relu(scale*x+shift)) per channel half with broadcast scale.
            agin = dr.tile([18, 1024], F32, tag="agin")
            gath = dr.tile([8, 18, 1024], F32, tag="gath")
            scb = dr.tile([1, 4], F32, tag="scb")
            nc.sync.dma_start(out=scb[:, 0:2], in_=scl3)
            nc.sync.dma_start(out=scb[:, 2:4], in_=sh3)
            ssb = cb.tile([16, 4], F32)
            nc.sync.dma_start(out=ssb, in_=scb.partition_broadcast(16))
            y3p = cb.tile([16, 1024], F32)
            y5 = y3.rearrange("p (gy ph py px gx) -> p gy ph py px gx",
                              gy=8, ph=2, py=2, px=4)
            for PY in range(4):
                for px in range(4):
                    kk = PY * 4 + px
                    for c in range(2):
                        nc.sync.dma_start(
                            out=y3p[kk:kk + 1, c * 512:(c + 1) * 512],
                            in_=y5[c:c + 1, :, PY // 2, PY % 2, px, :])
            for c in range(2):
                nc.scalar.activation(y3p[:, c * 512:(c + 1) * 512],
                                     y3p[:, c * 512:(c + 1) * 512],
                                     AF.Relu, bias=ssb[:, 2 + c:3 + c],
                                     scale=ssb[:, c:c + 1])
            Q = cb.tile([16, 1024], F32)
            nc.vector.tensor_mul(Q, y3p, y3p)
            ones16 = cb.tile([16, 1], F32)
            nc.vector.memset(ones16, 1.0)
            sqv = cb.tile([1, 1024], F32)
            for j in range(2):
                pq = bnps.tile([1, 512], F32, tag="bn")
                nc.tensor.matmul(pq, ones16, Q[:, j * 512:(j + 1) * 512],
                                 start=True, stop=True)
                nc.scalar.copy(sqv[:, j * 512:(j + 1) * 512], pq)
            ones1k = cb.tile([1, 1024], F32)
            nc.vector.memset(ones1k, 1.0)
            nc.sync.dma_start(out=agin[0:16, :], in_=y3p)
            nc.sync.dma_start(out=agin[16:17, :], in_=ones1k)
            nc.sync.dma_start(out=agin[17:18, :], in_=sqv)
            nc.gpsimd.collective_compute(
                "AllGather", ALU.bypass,
                replica_groups=[list(range(NCORES))],
                ins=[agin.opt()], outs=[gath.opt()])

        # ---------------- distance phase ----------------
        # Symmetric-triangle: c0 rows (m 0-3) compute all 16 col blocks; c1
        # rows (m 4-7) only the c1 half (n 8-15); host mirrors the lower-left
        # cross quadrant. agin rows are [p, 1, sq]; lhsT = [-2p, sq, 1] so
        # D^2 = -2 p_i.p_j + sq_i + sq_j in one K=18 f32r matmul.
        with tc.tile_pool(name="dist", bufs=1) as dist, \
             tc.tile_pool(name="stg", bufs=2) as stg, \
             tc.tile_pool(name="dps", bufs=8, space="PSUM") as dps:
            lhsT = dist.tile([128, 1024], F32R)
            nc.gpsimd.dma_start(out=lhsT[0:16, :], in_=agin[0:16, :])
            nc.gpsimd.dma_start(out=lhsT[16:17, :], in_=agin[17:18, :])
            nc.gpsimd.dma_start(out=lhsT[17:18, :], in_=agin[16:17, :])
            nc.vector.tensor_scalar_mul(lhsT[0:16, :], lhsT[0:16, :], -2.0)
            rhs = dist.tile([128, 8192], F32R)
            for c in range(2):
                nc.gpsimd.dma_start(
                    out=rhs[0:18, c * 4096:(c + 1) * 4096].rearrange(
                        "a (k n) -> a k n", k=8),
                    in_=gath[:, :, c * 512:(c + 1) * 512].rearrange(
                        "k a n -> a k n"))
            # replicate the 18 aug rows into 4 PE row-group strips so 4
            # K=18 matmuls run concurrently (tile_position row packing)
            for b in (32, 64, 96):
                nc.vector.tensor_copy(lhsT[b:b + 18, :], lhsT[0:18, :])
                nc.vector.tensor_copy(rhs[b:b + 18, :], rhs[0:18, :])
            ti = 0
            for m in range(8):
                nlist = range(16) if m < 4 else range(8, 16)
                stage = stg.tile([128, 8192], BF16, tag="stage")
                for n in nlist:
                    b = 32 * (ti % 4)
                    ti += 1
                    sl = stage[:, n * 512:(n + 1) * 512]
                    ps = dps.tile([128, 512], F32, tag="dp")
                    nc.tensor.matmul(ps,
                                     lhsT[b:b + 18, m * 128:(m + 1) * 128],
                                     rhs[b:b + 18, n * 512:(n + 1) * 512],
                                     start=True, stop=True,
                                     tile_position=(b, 0))
                    nc.vector.tensor_scalar_max(sl, ps, 0.0)
                    nc.scalar.activation(sl, sl, AF.Sqrt)
                c0 = 0 if m < 4 else 4096
                nc.sync.dma_start(out=out[m * 128:(m + 1) * 128, c0:8192],
                                  in_=stage[:, c0:8192])
    nc.finalize()
    return nc


def _prep_inputs(x, ws_, gs, bes):
    """Per-core numpy input dicts."""
    xp = np.pad(x[0], ((0, 0), (5, 5), (2, 3))).astype(np.float32)
    w0 = ws_[0]
    w0T = np.ascontiguousarray(
        w0.transpose(2, 3, 1, 0).reshape(27, 64)).astype(np.float32)
    wp, wsg = {}, {}
    for l in (1, 2):
        w = ws_[l]
        wp[l] = np.ascontiguousarray(np.stack(
            [np.concatenate([w[:, :, 0, p].T, w[:, :, 1, p].T], 0)
             for p in range(3)])).astype(np.float32)
        wsg[l] = np.ascontiguousarray(np.stack(
            [w[:, :, 2, p].T for p in range(3)])).astype(np.float32)
    w3 = ws_[3]
    wA3 = np.zeros((128, 66), np.float32)
    wB3 = np.zeros((64, 66), np.float32)
    for t in range(3):
        for ch in range(2):
            wA3[0:64, 32 * t + ch] = w3[ch, :, 0, t]
            wA3[64:128, 32 * t + ch] = w3[ch, :, 1, t]
            wB3[0:64, 32 * t + ch] = w3[ch, :, 2, t]
    g_all = np.concatenate([np.asarray(g, np.float32).ravel() for g in gs]
                           ).reshape(1, 194)
    be_all = np.concatenate([np.asarray(b, np.float32).ravel() for b in bes]
                            ).reshape(1, 194)
    in_maps = []
    for k in range(NCORES):
        col = np.empty((27, ROWS, WP), np.float32)
        for dy in range(3):
            for dx in range(3):
                for ci in range(3):
                    r0 = 32 * k + dy
                    col[(dy * 3 + dx) * 3 + ci] = xp[ci, r0:r0 + ROWS, dx:dx + WP]
        mask = np.zeros((8, WP), np.float32)
        for i, r in enumerate([0, 1, 2, 3, 36, 37, 38, 39]):
            ir = 32 * k - 4 + r
            if 0 <= ir < 256:
                mask[i, 1:257] = 1.0
        in_maps.append(dict(
            x0=np.ascontiguousarray(col.reshape(27, YFREE)),
            w0T=w0T, wp1=wp[1], ws1=wsg[1], wp2=wp[2], ws2=wsg[2],
            wA3=wA3, wB3=wB3, g_all=g_all, be_all=be_all,
            mask8=np.ascontiguousarray(mask.reshape(1, 8 * WP))))
    return in_maps


def kernel(x, w0, b0, g0, be0, w1, b1, g1, be1, w2, b2, g2, be2,
           w3, b3, g3, be3):
    # conv bias b_i cancels exactly inside BatchNorm (mean absorbs it); unused.
    if "nc" not in _CACHE:
        _CACHE["nc"] = build()
    nc = _CACHE["nc"]
    in_maps = _prep_inputs(
        np.asarray(x, np.float32),
        [np.asarray(w, np.float32) for w in (w0, w1, w2, w3)],
        (g0, g1, g2, g3), (be0, be1, be2, be3))
    res = run_bass_kernel_spmd(nc, in_maps, list(range(NCORES)))
    D = np.empty((8192, 8192), np.float32)
    for k in range(NCORES):
        o = res.results[k]["out"]  # [1024, 8192] bf16
        D[k * 512:(k + 1) * 512, :] = o[0:512, :].astype(np.float32)
        D[4096 + k * 512: 4096 + (k + 1) * 512, 4096:] = \
            o[512:1024, 4096:].astype(np.float32)
    D[4096:, :4096] = D[:4096, 4096:].T
    np.fill_diagonal(D, 0.0)   # exact: d(i,i)=0; device f32r rounding leaves ~0.1
    return D


# revision 25
# speedup vs baseline: 1.1464x; 1.0286x over previous
"""TRN2 Bass kernel for nn_KNN_model (conv stack + pairwise patch distances).

Strategy (8 NeuronCores, SPMD):
  - Convs sharded over H: each core computes a 40-row slab (32 owned + 4 halo
    each side) through all 4 conv+BN+ReLU layers in float32r (TF32-like) on PE.
    3x3 conv = 6 matmul streams per tile: 3 K=128 pairs (top+mid tap rows via a
    partition-shifted slab copy) + 3 K=64 singles (bottom tap row).
  - conv3 (2 out ch): wide-stationary form - one K=128 stream (top+mid rows,
    3 tap-cols x 2 ch = 6 psum rows) + one K=64 stream (bottom row) into the
    same [6,512] psum; DVE shift-adds combine the 3 tap columns.
  - BN stats: per-core partial (mean, var) via one 3D bn_stats/bn_aggr over
    owned rows, tiny AllGather + PE ones-matmul reduce; layers 0-2 transpose
    scale/shift to [C,1] via K=1 matmuls and apply fused in one ACT pass;
    layer 3 applies BN post-scatter on the [16,1024] patch tile.
  - Out-of-image halo rows are zeroed with a per-core mask input (SPMD-safe).
  - Patches [16,1024] + (ones, sq) rows AllGathered; D tile = one f32r K=18
    matmul per [128,512] (4-way tile_position row packing), DVE relu (bf16),
    ACT sqrt (bf16). Symmetric-triangle: c0 rows compute all 16 col blocks,
    c1 rows only the c1 half; host mirrors the lower-left quadrant, diag=0.
"""
import numpy as np
import ml_dtypes
import concourse.bacc as bacc
import concourse.bass as bass
import concourse.tile as tile
from concourse import mybir
from concourse.bass_utils import run_bass_kernel_spmd

F32 = mybir.dt.float32
F32R = mybir.dt.float32r
BF16 = mybir.dt.bfloat16
AF = mybir.ActivationFunctionType
ALU = mybir.AluOpType

NCORES = 8
WP = 258            # padded row width (256 + 2 pad cols)
ROWS = 40           # ext slab rows per core (32 owned + 4 halo each side)
LEAD = 4            # lead margin so tap offsets never go negative
HROWS = 42          # slab rows + 1 pad row top/bottom
HFREE = LEAD + HROWS * WP + 4   # 10844
YFREE = ROWS * WP   # 10320
EPS = 1e-5
GOFF = [0, 64, 128, 192]        # g/be packing offsets per layer
COUT = [64, 64, 64, 2]

_CACHE = {}


def _conv_tiles(s0=0, s1=YFREE):
    out, s = [], s0
    while s < s1:
        L = min(512, s1 - s)
        out.append((s, L))
        s += L
    return out


def _c3_tiles():
    # conv3 tiles: moving [q0, q0+n), outputs [q0+1, q0+n-1)
    q_lo, q_hi = 4 * WP, 36 * WP
    out, q0 = [], q_lo - 1
    while q0 + 1 < q_hi:
        n = min(512, q_hi - q0 + 1)
        out.append((q0, n))
        q0 += 510
    return out


def build():
    nc = bacc.Bacc(trn_type="TRN2", num_devices=NCORES)
    x0 = nc.dram_tensor("x0", [27, YFREE], F32, kind="ExternalInput").ap()
    w0T = nc.dram_tensor("w0T", [27, 64], F32, kind="ExternalInput").ap()
    wp_in, ws_in = {}, {}
    for l in (1, 2):
        wp_in[l] = nc.dram_tensor(f"wp{l}", [3, 128, 64], F32, kind="ExternalInput").ap()
        ws_in[l] = nc.dram_tensor(f"ws{l}", [3, 64, 64], F32, kind="ExternalInput").ap()
    wA3_in = nc.dram_tensor("wA3", [128, 66], F32, kind="ExternalInput").ap()
    wB3_in = nc.dram_tensor("wB3", [64, 66], F32, kind="ExternalInput").ap()
    g_all = nc.dram_tensor("g_all", [1, 194], F32, kind="ExternalInput").ap()
    be_all = nc.dram_tensor("be_all", [1, 194], F32, kind="ExternalInput").ap()
    mask8 = nc.dram_tensor("mask8", [1, 8 * WP], F32, kind="ExternalInput").ap()
    out = nc.dram_tensor("out", [1024, 8192], BF16, kind="ExternalOutput").ap()

    TILES = {0: _conv_tiles(WP, 39 * WP),
             1: _conv_tiles(2 * WP, 38 * WP),
             2: _conv_tiles(3 * WP, 37 * WP)}

    with tile.TileContext(nc) as tc:
      with tc.tile_pool(name="pers", bufs=1) as pers, \
           tc.tile_pool(name="dr", bufs=1, space="DRAM") as dr:
        gsb = pers.tile([1, 194], F32)
        nc.sync.dma_start(out=gsb, in_=g_all)
        besb = pers.tile([1, 194], F32)
        nc.sync.dma_start(out=besb, in_=be_all)
        ones1 = pers.tile([1, 1], F32)
        nc.vector.memset(ones1, 1.0)
        ones8 = pers.tile([8, 1], F32)
        nc.vector.memset(ones8, 0.125)   # 1/8 for mean-of-cores matmul
        epst = pers.tile([1, 1], F32)
        nc.vector.memset(epst, EPS)

        def bn_finish(l, C, regions, bnps, sbp, transpose=True):
            """Cross-core BN: partial stats -> AllGather -> scale/shift.

            Returns ([C,1] scale, [C,1] shift) when transpose else two
            [1,C] row vectors."""
            n = sum(r.shape[1] if r.ndim == 3 else 1 for r in regions)
            st = sbp.tile([C, n, 6], F32, tag=f"st{l}")
            i = 0
            for ap in regions:
                k = ap.shape[1] if ap.ndim == 3 else 1
                o = st[:, i:i + k, :] if ap.ndim == 3 else st[:, i, :]
                nc.vector.bn_stats(out=o, in_=ap)
                i += k
            mvt = sbp.tile([C, 2], F32, tag=f"mv{l}")
            nc.vector.bn_aggr(out=mvt, in_=st)
            sti = dr.tile([C, 2], F32, tag=f"sti{l}")
            sto = dr.tile([NCORES, C, 2], F32, tag=f"sto{l}")
            nc.gpsimd.dma_start(out=sti, in_=mvt)
            nc.gpsimd.collective_compute(
                "AllGather", ALU.bypass,
                replica_groups=[list(range(NCORES))],
                ins=[sti.opt()], outs=[sto.opt()])
            G = sbp.tile([8, 2 * C], F32, tag=f"G{l}")
            nc.sync.dma_start(out=G, in_=sto.rearrange("k c two -> k (c two)"))
            Gv = G.rearrange("p (c two) -> p c two", two=2)
            m2 = sbp.tile([8, C], F32, tag=f"m2{l}")
            nc.vector.tensor_mul(m2, Gv[:, :, 0], Gv[:, :, 0])
            pavg = bnps.tile([1, 2 * C], F32, tag="bn")
            nc.tensor.matmul(pavg, ones8, G, start=True, stop=True)
            pavg2 = bnps.tile([1, C], F32, tag="bn")
            nc.tensor.matmul(pavg2, ones8, m2, start=True, stop=True)
            A1 = sbp.tile([1, 2 * C], F32, tag=f"A1{l}")
            nc.scalar.copy(A1, pavg)
            A2 = sbp.tile([1, C], F32, tag=f"A2{l}")
            nc.scalar.copy(A2, pavg2)
            A1v = A1.rearrange("p (c two) -> p c two", two=2)
            am, av = A1v[:, :, 0], A1v[:, :, 1]
            t1 = sbp.tile([1, C], F32, tag=f"t1{l}")
            nc.vector.tensor_mul(t1, am, am)       # E[m]^2
            t2 = sbp.tile([1, C], F32, tag=f"t2{l}")
            nc.vector.tensor_sub(t2, A2, t1)       # Var(means)
            t3 = sbp.tile([1, C], F32, tag=f"t3{l}")
            nc.vector.tensor_add(t3, t2, av)       # + E[var] = total var
            sd = sbp.tile([1, C], F32, tag=f"sd{l}")
            nc.scalar.activation(sd, t3, AF.Sqrt, bias=epst)
            rs = sbp.tile([1, C], F32, tag=f"rs{l}")
            nc.vector.reciprocal(rs, sd)
            off = GOFF[l]
            scl = sbp.tile([1, C], F32, tag=f"scl{l}")
            nc.vector.tensor_mul(scl, gsb[:, off:off + C], rs)
            sh0 = sbp.tile([1, C], F32, tag=f"sh0{l}")
            nc.vector.tensor_mul(sh0, am, scl)
            sh = sbp.tile([1, C], F32, tag=f"sh{l}")
            nc.vector.tensor_sub(sh, besb[:, off:off + C], sh0)
            if not transpose:
                return scl, sh
            psc = bnps.tile([C, 1], F32, tag="bn")
            nc.tensor.matmul(psc, scl, ones1, start=True, stop=True)
            psh = bnps.tile([C, 1], F32, tag="bn")
            nc.tensor.matmul(psh, sh, ones1, start=True, stop=True)
            sbs = sbp.tile([C, 1], F32, tag=f"sbs{l}")
            nc.scalar.copy(sbs, psc)
            sbh = sbp.tile([C, 1], F32, tag=f"sbh{l}")
            nc.scalar.copy(sbh, psh)
            return sbs, sbh

        # ---------------- conv phase ----------------
        with tc.tile_pool(name="cb", bufs=1) as cb, \
             tc.tile_pool(name="hp", bufs=1) as hp, \
             tc.tile_pool(name="c3p", bufs=4) as c3p, \
             tc.tile_pool(name="cps", bufs=6, space="PSUM") as cps, \
             tc.tile_pool(name="bnps", bufs=2, space="PSUM") as bnps:
            x0t = cb.tile([27, YFREE], F32R)
            nc.gpsimd.dma_start(out=x0t, in_=x0)
            w0 = cb.tile([27, 64], F32R)
            nc.gpsimd.dma_start(out=w0, in_=w0T)
            wpair, wsing = {}, {}
            for l in (1, 2):
                wpl = cb.tile([128, 192], F32R, tag=f"twp{l}")
                nc.gpsimd.dma_start(
                    out=wpl.rearrange("p (t c) -> p t c", t=3),
                    in_=wp_in[l].rearrange("t p c -> p t c"))
                wsl = cb.tile([64, 192], F32R, tag=f"tws{l}")
                nc.gpsimd.dma_start(
                    out=wsl.rearrange("p (t c) -> p t c", t=3),
                    in_=ws_in[l].rearrange("t p c -> p t c"))
                for p in range(3):
                    wpair[(l, p)] = wpl[:, p * 64:(p + 1) * 64]
                    wsing[(l, p)] = wsl[:, p * 64:(p + 1) * 64]
            wA3 = cb.tile([128, 66], F32R)
            nc.gpsimd.dma_start(out=wA3, in_=wA3_in)
            wB3 = cb.tile([64, 66], F32R)
            nc.gpsimd.dma_start(out=wB3, in_=wB3_in)
            mskf = cb.tile([64, 8 * WP], F32)
            nc.sync.dma_start(out=mskf, in_=mask8.partition_broadcast(64))
            mv_ = mskf.rearrange("p (r c) -> p r c", c=WP)

            def finish_layer(l, y):
                """BN + ReLU + mask + build padded f32r slab with shifted copy."""
                yv = y.rearrange("p (r c) -> p r c", c=WP)
                regs = [yv[:, r, 1:257] for r in range(4, 36)]
                sbs, sbh = bn_finish(l, 64, regs, bnps, cb)
                h = hp.tile([128, HFREE], F32R, tag="h")
                nc.scalar.activation(h[0:64, LEAD + WP:LEAD + WP + YFREE], y,
                                     AF.Relu, bias=sbh, scale=sbs)
                hv = h[0:64, LEAD + WP:LEAD + WP + YFREE].rearrange(
                    "p (r c) -> p r c", c=WP)
                nc.vector.tensor_mul(hv[:, 0:4, :], hv[:, 0:4, :], mv_[:, 0:4, :])
                nc.vector.tensor_mul(hv[:, 36:40, :], hv[:, 36:40, :], mv_[:, 4:8, :])
                hcv = h[0:64, LEAD + WP:LEAD + WP + YFREE].rearrange(
                    "p (r c) -> p c r", c=WP)
                nc.vector.memset(hcv[:, 0, :].bitcast(F32), 0.0)
                nc.vector.memset(hcv[:, 257, :].bitcast(F32), 0.0)
                nc.vector.memset(h[0:64, 0:LEAD + WP].bitcast(F32), 0.0)
                nc.vector.memset(h[0:64, LEAD + WP + YFREE:HFREE].bitcast(F32), 0.0)
                nc.vector.tensor_copy(h[64:128, 0:HFREE - WP],
                                      h[0:64, WP:HFREE])
                nc.vector.memset(h[64:128, HFREE - WP:HFREE].bitcast(F32), 0.0)
                return h

            # conv0 (im2col input, K=27, one stream)
            if True:
                y = cb.tile([64, YFREE], F32, tag="y")
                for (s, L) in TILES[0]:
                    ps = cps.tile([64, 512], F32, tag="cps")
                    nc.tensor.matmul(ps[:, 0:L], w0, x0t[:, s:s + L],
                                     start=True, stop=True)
                    nc.scalar.copy(y[:, s:s + L], ps[:, 0:L])
                h = finish_layer(0, y)

            # conv1, conv2 (6 streams: 3 pairs K=128 + 3 singles K=64)
            GROUP = 6
            for l in (1, 2):
                y = cb.tile([64, YFREE], F32, tag="y")
                for g0 in range(0, len(TILES[l]), GROUP):
                    grp = TILES[l][g0:g0 + GROUP]
                    pss = [cps.tile([64, 512], F32, tag="cps", name=f"cps{g0}_{i}")
                           for i in range(len(grp))]
                    for p in range(3):
                        for ps, (s, L) in zip(pss, grp):
                            o = LEAD + 516 + s + p - 1
                            nc.tensor.matmul(ps[:, 0:L],
                                             wsing[(l, p)],
                                             h[0:64, o:o + L],
                                             start=(p == 0), stop=False)
                    for p in range(3):
                        for ps, (s, L) in zip(pss, grp):
                            o = LEAD + s + p - 1
                            nc.tensor.matmul(ps[:, 0:L],
                                             wpair[(l, p)],
                                             h[0:128, o:o + L],
                                             start=False, stop=(p == 2))
                    for ps, (s, L) in zip(pss, grp):
                        nc.scalar.copy(y[:, s:s + L], ps[:, 0:L])
                h = finish_layer(l, y)

            # conv3: wide-stationary, patch-order moving. Stream A (K=128):
            # rows (r-1, r), 6 stationary cols (3 tap-cols x 2 ch); stream B
            # (K=64): row r+1, accumulated into the same [6,512] psum. DVE
            # slice-adds combine the 3 tap columns (px/gx shifts, edges via
            # slice bounds = implicit zero pad). Output lands patch-major.
            def c3mov(p0, np_, off):
                wide = h[p0:p0 + np_, off:off + 2 * WP]
                w2 = wide.rearrange("p (py c) -> p py c", py=2)[:, :, 0:256]
                return w2.rearrange("p py (gx px) -> p py px gx", px=4)

            y3rf = cb.tile([64, YFREE], F32, tag="y")
            y3 = y3rf[0:2, 0:8192]
            for gy in range(8):
                for ph in range(2):
                    offA = LEAD + (4 + 4 * gy + 2 * ph) * WP + 1
                    ps3 = cps.tile([66, 512], F32, tag="cps",
                                   name=f"c3_{gy}_{ph}")
                    nc.tensor.matmul(ps3, wA3,
                                     c3mov(0, 128, offA),
                                     start=True, stop=False)
                    nc.tensor.matmul(ps3, wB3,
                                     c3mov(0, 64, offA + 2 * WP),
                                     start=False, stop=True)
                    t0 = (gy * 2 + ph) * 512
                    yt = y3[:, t0:t0 + 512].rearrange(
                        "p (py px gx) -> p py px gx", py=2, px=4)
                    sb3 = c3p.tile([2, 1536], F32, tag="c3sb",
                                   name=f"c3sb{gy}_{ph}")
                    for g in range(3):
                        nc.scalar.copy(sb3[:, g * 512:(g + 1) * 512],
                                       ps3[32 * g:32 * g + 2, :])
                    pv = sb3.rearrange("p (t py px gx) -> p t py px gx",
                                       t=3, py=2, px=4)
                    nc.vector.tensor_copy(yt[:, :, 0, :], pv[:, 1, :, 0, :])
                    nc.vector.tensor_add(yt[:, :, 1:4, :], pv[:, 1, :, 1:4, :],
                                         pv[:, 0, :, 0:3, :])
                    nc.vector.tensor_add(yt[:, :, 0, 1:], yt[:, :, 0, 1:],
                                         pv[:, 0, :, 3, 0:63])
                    nc.vector.tensor_add(yt[:, :, 0:3, :], yt[:, :, 0:3, :],
                                         pv[:, 2, :, 1:4, :])
                    nc.vector.tensor_add(yt[:, :, 3, 0:63], yt[:, :, 3, 0:63],
                                         pv[:, 2, :, 0, 1:64])
            scl3, sh3 = bn_finish(3, 2,
                                  [y3[:, i * 512:(i + 1) * 512]
                                   for i in range(16)], bnps, cb,
                                  transpose=False)

            # ---------------- patch build + AllGather ----------------
            # scatter raw conv3 rows -> patch-major [16,1024], then apply BN3
            # (relu(scale*x+shift)) per channel half with broadcast scale.
            agin = dr.tile([18, 1024], F32, tag="agin")
            gath = dr.tile([8, 18, 1024], F32, tag="gath")
            scb = dr.tile([1, 4], F32, tag="scb")
            nc.sync.dma_start(out=scb[:, 0:2], in_=scl3)
            nc.sync.dma_start(out=scb[:, 2:4], in_=sh3)
            ssb = cb.tile([16, 4], F32)
            nc.sync.dma_start(out=ssb, in_=scb.partition_broadcast(16))
            y3d = dr.tile([16, 1024], F32, tag="y3d")
            y3dr = y3d.rearrange("k (c gy gx) -> k c gy gx", c=2, gy=8)
            y5 = y3.rearrange("p (gy ph py px gx) -> p gy ph py px gx",
                              gy=8, ph=2, py=2, px=4)
            for PY in range(4):
                for px in range(4):
                    nc.sync.dma_start(out=y3dr[PY * 4 + px],
                                      in_=y5[:, :, PY // 2, PY % 2, px, :])
            y3p = cb.tile([16, 1024], F32)
            nc.sync.dma_start(out=y3p, in_=y3d)
            for c in range(2):
                nc.scalar.activation(y3p[:, c * 512:(c + 1) * 512],
                                     y3p[:, c * 512:(c + 1) * 512],
                                     AF.Relu, bias=ssb[:, 2 + c:3 + c],
                                     scale=ssb[:, c:c + 1])
            Q = cb.tile([16, 1024], F32)
            nc.vector.tensor_mul(Q, y3p, y3p)
            ones16 = cb.tile([16, 1], F32)
            nc.vector.memset(ones16, 1.0)
            sqv = cb.tile([1, 1024], F32)
            for j in range(2):
                pq = bnps.tile([1, 512], F32, tag="bn")
                nc.tensor.matmul(pq, ones16, Q[:, j * 512:(j + 1) * 512],
                                 start=True, stop=True)
                nc.scalar.copy(sqv[:, j * 512:(j + 1) * 512], pq)
            ones1k = cb.tile([1, 1024], F32)
            nc.vector.memset(ones1k, 1.0)
            nc.sync.dma_start(out=agin[0:16, :], in_=y3p)
            nc.sync.dma_start(out=agin[16:17, :], in_=ones1k)
            nc.sync.dma_start(out=agin[17:18, :], in_=sqv)
            nc.gpsimd.collective_compute(
                "AllGather", ALU.bypass,
                replica_groups=[list(range(NCORES))],
                ins=[agin.opt()], outs=[gath.opt()])

        # ---------------- distance phase ----------------
        # Symmetric-triangle: c0 rows (m 0-3) compute all 16 col blocks; c1
        # rows (m 4-7) only the c1 half (n 8-15); host mirrors the lower-left
        # cross quadrant. agin rows are [p, 1, sq]; lhsT = [-2p, sq, 1] so
        # D^2 = -2 p_i.p_j + sq_i + sq_j in one K=18 f32r matmul.
        with tc.tile_pool(name="dist", bufs=1) as dist, \
             tc.tile_pool(name="stg", bufs=2) as stg, \
             tc.tile_pool(name="dps", bufs=8, space="PSUM") as dps:
            lhsT = dist.tile([128, 1024], F32R)
            nc.gpsimd.dma_start(out=lhsT[0:16, :], in_=agin[0:16, :])
            nc.gpsimd.dma_start(out=lhsT[16:17, :], in_=agin[17:18, :])
            nc.gpsimd.dma_start(out=lhsT[17:18, :], in_=agin[16:17, :])
            nc.vector.tensor_scalar_mul(lhsT[0:16, :], lhsT[0:16, :], -2.0)
            rhs = dist.tile([128, 8192], F32R)
            for c in range(2):
                nc.gpsimd.dma_start(
                    out=rhs[0:18, c * 4096:(c + 1) * 4096].rearrange(
                        "a (k n) -> a k n", k=8),
                    in_=gath[:, :, c * 512:(c + 1) * 512].rearrange(
                        "k a n -> a k n"))
            # replicate the 18 aug rows into 4 PE row-group strips so 4
            # K=18 matmuls run concurrently (tile_position row packing)
            for b in (32, 64, 96):
                nc.vector.tensor_copy(lhsT[b:b + 18, :], lhsT[0:18, :])
                nc.vector.tensor_copy(rhs[b:b + 18, :], rhs[0:18, :])
            ti = 0
            for m in range(8):
                nlist = range(16) if m < 4 else range(8, 16)
                stage = stg.tile([128, 8192], BF16, tag="stage")
                for n in nlist:
                    b = 32 * (ti % 4)
                    ti += 1
                    sl = stage[:, n * 512:(n + 1) * 512]
                    ps = dps.tile([128, 512], F32, tag="dp")
                    nc.tensor.matmul(ps,
                                     lhsT[b:b + 18, m * 128:(m + 1) * 128],
                                     rhs[b:b + 18, n * 512:(n + 1) * 512],
                                     start=True, stop=True,
                                     tile_position=(b, 0))
                    nc.vector.tensor_scalar_max(sl, ps, 0.0)
                    nc.scalar.activation(sl, sl, AF.Sqrt)
                c0 = 0 if m < 4 else 4096
                nc.sync.dma_start(out=out[m * 128:(m + 1) * 128, c0:8192],
                                  in_=stage[:, c0:8192])
    nc.finalize()
    return nc


def _prep_inputs(x, ws_, gs, bes):
    """Per-core numpy input dicts."""
    xp = np.pad(x[0], ((0, 0), (5, 5), (2, 3))).astype(np.float32)
    w0 = ws_[0]
    w0T = np.ascontiguousarray(
        w0.transpose(2, 3, 1, 0).reshape(27, 64)).astype(np.float32)
    wp, wsg = {}, {}
    for l in (1, 2):
        w = ws_[l]
        wp[l] = np.ascontiguousarray(np.stack(
            [np.concatenate([w[:, :, 0, p].T, w[:, :, 1, p].T], 0)
             for p in range(3)])).astype(np.float32)
        wsg[l] = np.ascontiguousarray(np.stack(
            [w[:, :, 2, p].T for p in range(3)])).astype(np.float32)
    w3 = ws_[3]
    wA3 = np.zeros((128, 66), np.float32)
    wB3 = np.zeros((64, 66), np.float32)
    for t in range(3):
        for ch in range(2):
            wA3[0:64, 32 * t + ch] = w3[ch, :, 0, t]
            wA3[64:128, 32 * t + ch] = w3[ch, :, 1, t]
            wB3[0:64, 32 * t + ch] = w3[ch, :, 2, t]
    g_all = np.concatenate([np.asarray(g, np.float32).ravel() for g in gs]
                           ).reshape(1, 194)
    be_all = np.concatenate([np.asarray(b, np.float32).ravel() for b in bes]
                            ).reshape(1, 194)
    in_maps = []
    for k in range(NCORES):
        col = np.empty((27, ROWS, WP), np.float32)
        for dy in range(3):
            for dx in range(3):
                for ci in range(3):
                    r0 = 32 * k + dy
                    col[(dy * 3 + dx) * 3 + ci] = xp[ci, r0:r0 + ROWS, dx:dx + WP]
        mask = np.zeros((8, WP), np.float32)
        for i, r in enumerate([0, 1, 2, 3, 36, 37, 38, 39]):
            ir = 32 * k - 4 + r
            if 0 <= ir < 256:
                mask[i, 1:257] = 1.0
        in_maps.append(dict(
            x0=np.ascontiguousarray(col.reshape(27, YFREE)),
            w0T=w0T, wp1=wp[1], ws1=wsg[1], wp2=wp[2], ws2=wsg[2],
            wA3=wA3, wB3=wB3, g_all=g_all, be_all=be_all,
            mask8=np.ascontiguousarray(mask.reshape(1, 8 * WP))))
    return in_maps


def kernel(x, w0, b0, g0, be0, w1, b1, g1, be1, w2, b2, g2, be2,
           w3, b3, g3, be3):
    # conv bias b_i cancels exactly inside BatchNorm (mean absorbs it); unused.
    if "nc" not in _CACHE:
        _CACHE["nc"] = build()
    nc = _CACHE["nc"]
    in_maps = _prep_inputs(
        np.asarray(x, np.float32),
        [np.asarray(w, np.float32) for w in (w0, w1, w2, w3)],
        (g0, g1, g2, g3), (be0, be1, be2, be3))
    res = run_bass_kernel_spmd(nc, in_maps, list(range(NCORES)))
    D = np.empty((8192, 8192), np.float32)
    for k in range(NCORES):
        o = res.results[k]["out"]  # [1024, 8192] bf16
        D[k * 512:(k + 1) * 512, :] = o[0:512, :].astype(np.float32)
        D[4096 + k * 512: 4096 + (k + 1) * 512, 4096:] = \
            o[512:1024, 4096:].astype(np.float32)
    D[4096:, :4096] = D[:4096, 4096:].T
    np.fill_diagonal(D, 0.0)   # exact: d(i,i)=0; device f32r rounding leaves ~0.1
    return D


# revision 26
# speedup vs baseline: 1.1768x; 1.0265x over previous
"""TRN2 Bass kernel for nn_KNN_model (conv stack + pairwise patch distances).

Strategy (8 NeuronCores, SPMD):
  - Convs sharded over H: each core computes a 40-row slab (32 owned + 4 halo
    each side) through all 4 conv+BN+ReLU layers in float32r (TF32-like) on PE.
    3x3 conv = 6 matmul streams per tile: 3 K=128 pairs (top+mid tap rows via a
    partition-shifted slab copy) + 3 K=64 singles (bottom tap row).
  - conv3 (2 out ch): wide-stationary form - one K=128 stream (top+mid rows,
    3 tap-cols x 2 ch = 6 psum rows) + one K=64 stream (bottom row) into the
    same [6,512] psum; DVE shift-adds combine the 3 tap columns.
  - BN stats: per-core partial (mean, var) via one 3D bn_stats/bn_aggr over
    owned rows, tiny AllGather + PE ones-matmul reduce; layers 0-2 transpose
    scale/shift to [C,1] via K=1 matmuls and apply fused in one ACT pass;
    layer 3 applies BN post-scatter on the [16,1024] patch tile.
  - Out-of-image halo rows are zeroed with a per-core mask input (SPMD-safe).
  - Patches [16,1024] + (ones, sq) rows AllGathered; D tile = one f32r K=18
    matmul per [128,512] (4-way tile_position row packing), DVE relu (bf16),
    ACT sqrt (bf16). Symmetric-triangle: c0 rows compute all 16 col blocks,
    c1 rows only the c1 half; host mirrors the lower-left quadrant, diag=0.
"""
import numpy as np
import ml_dtypes
import concourse.bacc as bacc
import concourse.bass as bass
import concourse.tile as tile
from concourse import mybir
from concourse.bass_utils import run_bass_kernel_spmd

F32 = mybir.dt.float32
F32R = mybir.dt.float32r
BF16 = mybir.dt.bfloat16
AF = mybir.ActivationFunctionType
ALU = mybir.AluOpType

NCORES = 8
WP = 258            # padded row width (256 + 2 pad cols)
ROWS = 40           # ext slab rows per core (32 owned + 4 halo each side)
LEAD = 4            # lead margin so tap offsets never go negative
HROWS = 42          # slab rows + 1 pad row top/bottom
HFREE = LEAD + HROWS * WP + 4   # 10844
YFREE = ROWS * WP   # 10320
EPS = 1e-5
GOFF = [0, 64, 128, 192]        # g/be packing offsets per layer
COUT = [64, 64, 64, 2]

_CACHE = {}


def _conv_tiles(s0=0, s1=YFREE):
    out, s = [], s0
    while s < s1:
        L = min(512, s1 - s)
        out.append((s, L))
        s += L
    return out


def _c3_tiles():
    # conv3 tiles: moving [q0, q0+n), outputs [q0+1, q0+n-1)
    q_lo, q_hi = 4 * WP, 36 * WP
    out, q0 = [], q_lo - 1
    while q0 + 1 < q_hi:
        n = min(512, q_hi - q0 + 1)
        out.append((q0, n))
        q0 += 510
    return out


def build():
    nc = bacc.Bacc(trn_type="TRN2", num_devices=NCORES)
    x0 = nc.dram_tensor("x0", [27, YFREE], F32, kind="ExternalInput").ap()
    w0T = nc.dram_tensor("w0T", [27, 64], F32, kind="ExternalInput").ap()
    wp_in, ws_in = {}, {}
    for l in (1, 2):
        wp_in[l] = nc.dram_tensor(f"wp{l}", [3, 128, 64], F32, kind="ExternalInput").ap()
        ws_in[l] = nc.dram_tensor(f"ws{l}", [3, 64, 64], F32, kind="ExternalInput").ap()
    wA3_in = nc.dram_tensor("wA3", [128, 66], F32, kind="ExternalInput").ap()
    wB3_in = nc.dram_tensor("wB3", [64, 66], F32, kind="ExternalInput").ap()
    g_col = nc.dram_tensor("g_col", [64, 4], F32, kind="ExternalInput").ap()
    be_col = nc.dram_tensor("be_col", [64, 4], F32, kind="ExternalInput").ap()
    mask8 = nc.dram_tensor("mask8", [1, 8 * WP], F32, kind="ExternalInput").ap()
    out = nc.dram_tensor("out", [1024, 8192], BF16, kind="ExternalOutput").ap()

    TILES = {0: _conv_tiles(WP, 39 * WP),
             1: _conv_tiles(2 * WP, 38 * WP),
             2: _conv_tiles(3 * WP, 37 * WP)}

    with tile.TileContext(nc) as tc:
      with tc.tile_pool(name="pers", bufs=1) as pers, \
           tc.tile_pool(name="dr", bufs=1, space="DRAM") as dr:
        gsb = pers.tile([64, 4], F32)
        nc.sync.dma_start(out=gsb, in_=g_col)
        besb = pers.tile([64, 4], F32)
        nc.sync.dma_start(out=besb, in_=be_col)
        epsC = pers.tile([64, 1], F32)
        nc.vector.memset(epsC, EPS)

        def bn_finish(l, C, regions, bnps, sbp, transpose=True):
            """Cross-core BN: partial stats -> AllGather -> [C,1] scale/shift
            computed directly with free-dim reductions (no PE transposes)."""
            n = sum(r.shape[1] if r.ndim == 3 else 1 for r in regions)
            st = sbp.tile([C, n, 6], F32, tag=f"st{l}")
            i = 0
            for ap in regions:
                k = ap.shape[1] if ap.ndim == 3 else 1
                o = st[:, i:i + k, :] if ap.ndim == 3 else st[:, i, :]
                nc.vector.bn_stats(out=o, in_=ap)
                i += k
            mvt = sbp.tile([C, 2], F32, tag=f"mv{l}")
            nc.vector.bn_aggr(out=mvt, in_=st)
            sti = dr.tile([C, 2], F32, tag=f"sti{l}")
            sto = dr.tile([NCORES, C, 2], F32, tag=f"sto{l}")
            nc.gpsimd.dma_start(out=sti, in_=mvt)
            nc.gpsimd.collective_compute(
                "AllGather", ALU.bypass,
                replica_groups=[list(range(NCORES))],
                ins=[sti.opt()], outs=[sto.opt()])
            Gt = sbp.tile([C, 8, 2], F32, tag=f"Gt{l}")
            nc.sync.dma_start(out=Gt, in_=sto.rearrange("k c two -> c k two"))
            m, v = Gt[:, :, 0], Gt[:, :, 1]
            m2 = sbp.tile([C, 8], F32, tag=f"m2{l}")
            nc.vector.tensor_mul(m2, m, m)
            X = mybir.AxisListType.X
            S1 = sbp.tile([C, 1], F32, tag=f"S1{l}")
            nc.vector.tensor_reduce(S1, m, X, ALU.add)
            S2 = sbp.tile([C, 1], F32, tag=f"S2{l}")
            nc.vector.tensor_reduce(S2, v, X, ALU.add)
            S3 = sbp.tile([C, 1], F32, tag=f"S3{l}")
            nc.vector.tensor_reduce(S3, m2, X, ALU.add)
            am = sbp.tile([C, 1], F32, tag=f"am{l}")
            nc.vector.tensor_scalar_mul(am, S1, 0.125)
            t1 = sbp.tile([C, 1], F32, tag=f"t1{l}")
            nc.vector.tensor_mul(t1, am, am)        # E[m]^2
            t2 = sbp.tile([C, 1], F32, tag=f"t2{l}")
            nc.vector.tensor_add(t2, S2, S3)
            t3 = sbp.tile([C, 1], F32, tag=f"t3{l}")
            nc.vector.tensor_scalar_mul(t3, t2, 0.125)  # E[var] + E[m^2]
            t4 = sbp.tile([C, 1], F32, tag=f"t4{l}")
            nc.vector.tensor_sub(t4, t3, t1)        # total var
            sd = sbp.tile([C, 1], F32, tag=f"sd{l}")
            nc.scalar.activation(sd, t4, AF.Sqrt, bias=epsC[0:C])
            rs = sbp.tile([C, 1], F32, tag=f"rs{l}")
            nc.vector.reciprocal(rs, sd)
            scl = sbp.tile([C, 1], F32, tag=f"scl{l}")
            nc.vector.tensor_mul(scl, gsb[0:C, l:l + 1], rs)
            sh0 = sbp.tile([C, 1], F32, tag=f"sh0{l}")
            nc.vector.tensor_mul(sh0, am, scl)
            sh = sbp.tile([C, 1], F32, tag=f"sh{l}")
            nc.vector.tensor_sub(sh, besb[0:C, l:l + 1], sh0)
            return scl, sh

        # ---------------- conv phase ----------------
        with tc.tile_pool(name="cb", bufs=1) as cb, \
             tc.tile_pool(name="hp", bufs=1) as hp, \
             tc.tile_pool(name="c3p", bufs=4) as c3p, \
             tc.tile_pool(name="cps", bufs=6, space="PSUM") as cps, \
             tc.tile_pool(name="bnps", bufs=2, space="PSUM") as bnps:
            w0 = cb.tile([27, 64], F32R)
            nc.gpsimd.dma_start(out=w0, in_=w0T)
            x0t = cb.tile([27, YFREE], F32R)
            nc.gpsimd.dma_start(out=x0t, in_=x0)
            wpair, wsing = {}, {}
            for l in (1, 2):
                wpl = cb.tile([128, 192], F32R, tag=f"twp{l}")
                nc.gpsimd.dma_start(
                    out=wpl.rearrange("p (t c) -> p t c", t=3),
                    in_=wp_in[l].rearrange("t p c -> p t c"))
                wsl = cb.tile([64, 192], F32R, tag=f"tws{l}")
                nc.gpsimd.dma_start(
                    out=wsl.rearrange("p (t c) -> p t c", t=3),
                    in_=ws_in[l].rearrange("t p c -> p t c"))
                for p in range(3):
                    wpair[(l, p)] = wpl[:, p * 64:(p + 1) * 64]
                    wsing[(l, p)] = wsl[:, p * 64:(p + 1) * 64]
            mskf = cb.tile([64, 8 * WP], F32)
            nc.sync.dma_start(out=mskf, in_=mask8.partition_broadcast(64))
            mv_ = mskf.rearrange("p (r c) -> p r c", c=WP)
            wA3 = cb.tile([128, 66], F32R)
            nc.gpsimd.dma_start(out=wA3, in_=wA3_in)
            wB3 = cb.tile([64, 66], F32R)
            nc.gpsimd.dma_start(out=wB3, in_=wB3_in)

            def finish_layer(l, y):
                """BN + ReLU + mask + build padded f32r slab with shifted copy."""
                yv = y.rearrange("p (r c) -> p r c", c=WP)
                regs = [yv[:, r, 1:257] for r in range(4, 36)]
                sbs, sbh = bn_finish(l, 64, regs, bnps, cb)
                h = hp.tile([128, HFREE], F32R, tag="h")
                nc.scalar.activation(h[0:64, LEAD + WP:LEAD + WP + YFREE], y,
                                     AF.Relu, bias=sbh, scale=sbs)
                hv = h[0:64, LEAD + WP:LEAD + WP + YFREE].rearrange(
                    "p (r c) -> p r c", c=WP)
                nc.vector.tensor_mul(hv[:, 0:4, :], hv[:, 0:4, :], mv_[:, 0:4, :])
                nc.vector.tensor_mul(hv[:, 36:40, :], hv[:, 36:40, :], mv_[:, 4:8, :])
                hcv = h[0:64, LEAD + WP:LEAD + WP + YFREE].rearrange(
                    "p (r c) -> p c r", c=WP)
                nc.vector.memset(hcv[:, 0, :].bitcast(F32), 0.0)
                nc.vector.memset(hcv[:, 257, :].bitcast(F32), 0.0)
                nc.vector.memset(h[0:64, 0:LEAD + WP].bitcast(F32), 0.0)
                nc.vector.memset(h[0:64, LEAD + WP + YFREE:HFREE].bitcast(F32), 0.0)
                nc.vector.tensor_copy(h[64:128, 0:HFREE - WP],
                                      h[0:64, WP:HFREE])
                nc.vector.memset(h[64:128, HFREE - WP:HFREE].bitcast(F32), 0.0)
                return h

            # conv0 (im2col input, K=27, one stream)
            if True:
                y = cb.tile([64, YFREE], F32, tag="y")
                for (s, L) in TILES[0]:
                    ps = cps.tile([64, 512], F32, tag="cps")
                    nc.tensor.matmul(ps[:, 0:L], w0, x0t[:, s:s + L],
                                     start=True, stop=True)
                    nc.scalar.copy(y[:, s:s + L], ps[:, 0:L])
                h = finish_layer(0, y)

            # conv1, conv2 (6 streams: 3 pairs K=128 + 3 singles K=64)
            GROUP = 6
            for l in (1, 2):
                y = cb.tile([64, YFREE], F32, tag="y")
                for g0 in range(0, len(TILES[l]), GROUP):
                    grp = TILES[l][g0:g0 + GROUP]
                    pss = [cps.tile([64, 512], F32, tag="cps", name=f"cps{g0}_{i}")
                           for i in range(len(grp))]
                    for p in range(3):
                        for ps, (s, L) in zip(pss, grp):
                            o = LEAD + 516 + s + p - 1
                            nc.tensor.matmul(ps[:, 0:L],
                                             wsing[(l, p)],
                                             h[0:64, o:o + L],
                                             start=(p == 0), stop=False)
                    for p in range(3):
                        for ps, (s, L) in zip(pss, grp):
                            o = LEAD + s + p - 1
                            nc.tensor.matmul(ps[:, 0:L],
                                             wpair[(l, p)],
                                             h[0:128, o:o + L],
                                             start=False, stop=(p == 2))
                    for ps, (s, L) in zip(pss, grp):
                        nc.scalar.copy(y[:, s:s + L], ps[:, 0:L])
                h = finish_layer(l, y)

            # conv3: wide-stationary, patch-order moving. Stream A (K=128):
            # rows (r-1, r), 6 stationary cols (3 tap-cols x 2 ch); stream B
            # (K=64): row r+1, accumulated into the same [6,512] psum. DVE
            # slice-adds combine the 3 tap columns (px/gx shifts, edges via
            # slice bounds = implicit zero pad). Output lands patch-major.
            def c3mov(p0, np_, off):
                wide = h[p0:p0 + np_, off:off + 2 * WP]
                w2 = wide.rearrange("p (py c) -> p py c", py=2)[:, :, 0:256]
                return w2.rearrange("p py (gx px) -> p py px gx", px=4)

            y3rf = cb.tile([64, YFREE], F32, tag="y")
            y3 = y3rf[0:2, 0:8192]
            for gy in range(8):
                for ph in range(2):
                    offA = LEAD + (4 + 4 * gy + 2 * ph) * WP + 1
                    ps3 = cps.tile([66, 512], F32, tag="cps",
                                   name=f"c3_{gy}_{ph}")
                    nc.tensor.matmul(ps3, wA3,
                                     c3mov(0, 128, offA),
                                     start=True, stop=False)
                    nc.tensor.matmul(ps3, wB3,
                                     c3mov(0, 64, offA + 2 * WP),
                                     start=False, stop=True)
                    t0 = (gy * 2 + ph) * 512
                    yt = y3[:, t0:t0 + 512].rearrange(
                        "p (py px gx) -> p py px gx", py=2, px=4)
                    nc.scalar.copy(y3[:, t0:t0 + 512], ps3[32:34, :])
                    p0 = ps3[0:2, :].rearrange("q (py px gx) -> q py px gx",
                                               py=2, px=4)
                    p2 = ps3[64:66, :].rearrange("q (py px gx) -> q py px gx",
                                                 py=2, px=4)
                    nc.vector.tensor_add(yt[:, :, 1:4, :], yt[:, :, 1:4, :],
                                         p0[:, :, 0:3, :])
                    nc.vector.tensor_add(yt[:, :, 0, 1:], yt[:, :, 0, 1:],
                                         p0[:, :, 3, 0:63])
                    nc.vector.tensor_add(yt[:, :, 0:3, :], yt[:, :, 0:3, :],
                                         p2[:, :, 1:4, :])
                    nc.vector.tensor_add(yt[:, :, 3, 0:63], yt[:, :, 3, 0:63],
                                         p2[:, :, 0, 1:64])
            scl3, sh3 = bn_finish(3, 2,
                                  [y3[:, i * 512:(i + 1) * 512]
                                   for i in range(16)], bnps, cb,
                                  transpose=False)

            # ---------------- patch build + AllGather ----------------
            # scatter raw conv3 rows -> patch-major [16,1024], then apply BN3
            # (relu(scale*x+shift)) per channel half with broadcast scale.
            agin = dr.tile([18, 1024], F32, tag="agin")
            gath = dr.tile([8, 18, 1024], F32, tag="gath")
            scb = dr.tile([1, 4], F32, tag="scb")
            nc.sync.dma_start(out=scb[:, 0:2], in_=scl3)
            nc.sync.dma_start(out=scb[:, 2:4], in_=sh3)
            ssb = cb.tile([16, 4], F32)
            nc.sync.dma_start(out=ssb, in_=scb.partition_broadcast(16))
            y3d = dr.tile([16, 1024], F32, tag="y3d")
            y3dr = y3d.rearrange("k (c gy gx) -> k c gy gx", c=2, gy=8)
            y5 = y3.rearrange("p (gy ph py px gx) -> p gy ph py px gx",
                              gy=8, ph=2, py=2, px=4)
            for PY in range(4):
                for px in range(4):
                    nc.sync.dma_start(out=y3dr[PY * 4 + px],
                                      in_=y5[:, :, PY // 2, PY % 2, px, :])
            y3p = cb.tile([16, 1024], F32)
            nc.sync.dma_start(out=y3p, in_=y3d)
            for c in range(2):
                nc.scalar.activation(y3p[:, c * 512:(c + 1) * 512],
                                     y3p[:, c * 512:(c + 1) * 512],
                                     AF.Relu, bias=ssb[:, 2 + c:3 + c],
                                     scale=ssb[:, c:c + 1])
            Q = cb.tile([16, 1024], F32)
            nc.vector.tensor_mul(Q, y3p, y3p)
            ones16 = cb.tile([16, 1], F32)
            nc.vector.memset(ones16, 1.0)
            sqv = cb.tile([1, 1024], F32)
            for j in range(2):
                pq = bnps.tile([1, 512], F32, tag="bn")
                nc.tensor.matmul(pq, ones16, Q[:, j * 512:(j + 1) * 512],
                                 start=True, stop=True)
                nc.scalar.copy(sqv[:, j * 512:(j + 1) * 512], pq)
            ones1k = cb.tile([1, 1024], F32)
            nc.vector.memset(ones1k, 1.0)
            nc.sync.dma_start(out=agin[0:16, :], in_=y3p)
            nc.sync.dma_start(out=agin[16:17, :], in_=ones1k)
            nc.sync.dma_start(out=agin[17:18, :], in_=sqv)
            nc.gpsimd.collective_compute(
                "AllGather", ALU.bypass,
                replica_groups=[list(range(NCORES))],
                ins=[agin.opt()], outs=[gath.opt()])

        # ---------------- distance phase ----------------
        # Symmetric-triangle: c0 rows (m 0-3) compute all 16 col blocks; c1
        # rows (m 4-7) only the c1 half (n 8-15); host mirrors the lower-left
        # cross quadrant. agin rows are [p, 1, sq]; lhsT = [-2p, sq, 1] so
        # D^2 = -2 p_i.p_j + sq_i + sq_j in one K=18 f32r matmul.
        with tc.tile_pool(name="dist", bufs=1) as dist, \
             tc.tile_pool(name="stg", bufs=2) as stg, \
             tc.tile_pool(name="dps", bufs=8, space="PSUM") as dps:
            lhsT = dist.tile([128, 1024], F32R)
            nc.gpsimd.dma_start(out=lhsT[0:16, :], in_=agin[0:16, :])
            nc.gpsimd.dma_start(out=lhsT[16:17, :], in_=agin[17:18, :])
            nc.gpsimd.dma_start(out=lhsT[17:18, :], in_=agin[16:17, :])
            nc.vector.tensor_scalar_mul(lhsT[0:16, :], lhsT[0:16, :], -2.0)
            rhs = dist.tile([128, 8192], F32R)
            for c in range(2):
                nc.gpsimd.dma_start(
                    out=rhs[0:18, c * 4096:(c + 1) * 4096].rearrange(
                        "a (k n) -> a k n", k=8),
                    in_=gath[:, :, c * 512:(c + 1) * 512].rearrange(
                        "k a n -> a k n"))
            # replicate the 18 aug rows into 4 PE row-group strips so 4
            # K=18 matmuls run concurrently (tile_position row packing)
            for b in (32, 64, 96):
                nc.vector.tensor_copy(lhsT[b:b + 18, :], lhsT[0:18, :])
                nc.vector.tensor_copy(rhs[b:b + 18, :], rhs[0:18, :])
            ti = 0
            for m in range(8):
                nlist = range(16) if m < 4 else range(8, 16)
                stage = stg.tile([128, 8192], BF16, tag="stage")
                for n in nlist:
                    b = 32 * (ti % 4)
                    ti += 1
                    sl = stage[:, n * 512:(n + 1) * 512]
                    ps = dps.tile([128, 512], F32, tag="dp")
                    nc.tensor.matmul(ps,
                                     lhsT[b:b + 18, m * 128:(m + 1) * 128],
                                     rhs[b:b + 18, n * 512:(n + 1) * 512],
                                     start=True, stop=True,
                                     tile_position=(b, 0))
                    nc.vector.tensor_scalar_max(sl, ps, 0.0)
                    nc.scalar.activation(sl, sl, AF.Sqrt)
                c0 = 0 if m < 4 else 4096
                nc.sync.dma_start(out=out[m * 128:(m + 1) * 128, c0:8192],
                                  in_=stage[:, c0:8192])
    nc.finalize()
    return nc


def _prep_inputs(x, ws_, gs, bes):
    """Per-core numpy input dicts."""
    xp = np.pad(x[0], ((0, 0), (5, 5), (2, 3))).astype(np.float32)
    w0 = ws_[0]
    w0T = np.ascontiguousarray(
        w0.transpose(2, 3, 1, 0).reshape(27, 64)).astype(np.float32)
    wp, wsg = {}, {}
    for l in (1, 2):
        w = ws_[l]
        wp[l] = np.ascontiguousarray(np.stack(
            [np.concatenate([w[:, :, 0, p].T, w[:, :, 1, p].T], 0)
             for p in range(3)])).astype(np.float32)
        wsg[l] = np.ascontiguousarray(np.stack(
            [w[:, :, 2, p].T for p in range(3)])).astype(np.float32)
    w3 = ws_[3]
    wA3 = np.zeros((128, 66), np.float32)
    wB3 = np.zeros((64, 66), np.float32)
    for t in range(3):
        for ch in range(2):
            wA3[0:64, 32 * t + ch] = w3[ch, :, 0, t]
            wA3[64:128, 32 * t + ch] = w3[ch, :, 1, t]
            wB3[0:64, 32 * t + ch] = w3[ch, :, 2, t]
    g_col = np.zeros((64, 4), np.float32)
    be_col = np.zeros((64, 4), np.float32)
    for l in range(4):
        g_col[0:COUT[l], l] = np.asarray(gs[l], np.float32).ravel()
        be_col[0:COUT[l], l] = np.asarray(bes[l], np.float32).ravel()
    in_maps = []
    for k in range(NCORES):
        col = np.empty((27, ROWS, WP), np.float32)
        for dy in range(3):
            for dx in range(3):
                for ci in range(3):
                    r0 = 32 * k + dy
                    col[(dy * 3 + dx) * 3 + ci] = xp[ci, r0:r0 + ROWS, dx:dx + WP]
        mask = np.zeros((8, WP), np.float32)
        for i, r in enumerate([0, 1, 2, 3, 36, 37, 38, 39]):
            ir = 32 * k - 4 + r
            if 0 <= ir < 256:
                mask[i, 1:257] = 1.0
        in_maps.append(dict(
            x0=np.ascontiguousarray(col.reshape(27, YFREE)),
            w0T=w0T, wp1=wp[1], ws1=wsg[1], wp2=wp[2], ws2=wsg[2],
            wA3=wA3, wB3=wB3, g_col=g_col, be_col=be_col,
            mask8=np.ascontiguousarray(mask.reshape(1, 8 * WP))))
    return in_maps


def kernel(x, w0, b0, g0, be0, w1, b1, g1, be1, w2, b2, g2, be2,
           w3, b3, g3, be3):
    # conv bias b_i cancels exactly inside BatchNorm (mean absorbs it); unused.
    if "nc" not in _CACHE:
        _CACHE["nc"] = build()
    nc = _CACHE["nc"]
    in_maps = _prep_inputs(
        np.asarray(x, np.float32),
        [np.asarray(w, np.float32) for w in (w0, w1, w2, w3)],
        (g0, g1, g2, g3), (be0, be1, be2, be3))
    res = run_bass_kernel_spmd(nc, in_maps, list(range(NCORES)))
    D = np.empty((8192, 8192), np.float32)
    for k in range(NCORES):
        o = res.results[k]["out"]  # [1024, 8192] bf16
        D[k * 512:(k + 1) * 512, :] = o[0:512, :].astype(np.float32)
        D[4096 + k * 512: 4096 + (k + 1) * 512, 4096:] = \
            o[512:1024, 4096:].astype(np.float32)
    D[4096:, :4096] = D[:4096, 4096:].T
    np.fill_diagonal(D, 0.0)   # exact: d(i,i)=0; device f32r rounding leaves ~0.1
    return D


# revision 27
# speedup vs baseline: 1.2135x; 1.0312x over previous
"""TRN2 Bass kernel for nn_KNN_model (conv stack + pairwise patch distances).

Strategy (8 NeuronCores, SPMD):
  - Convs sharded over H: each core computes a 40-row slab (32 owned + 4 halo
    each side) through all 4 conv+BN+ReLU layers in float32r (TF32-like) on PE.
    3x3 conv = 6 matmul streams per tile: 3 K=128 pairs (top+mid tap rows via a
    partition-shifted slab copy) + 3 K=64 singles (bottom tap row).
  - conv3 (2 out ch): wide-stationary form - one K=128 stream (top+mid rows,
    3 tap-cols x 2 ch = 6 psum rows) + one K=64 stream (bottom row) into the
    same [6,512] psum; DVE shift-adds combine the 3 tap columns.
  - BN stats: per-core partial (mean, var) via one 3D bn_stats/bn_aggr over
    owned rows, tiny AllGather + PE ones-matmul reduce; layers 0-2 transpose
    scale/shift to [C,1] via K=1 matmuls and apply fused in one ACT pass;
    layer 3 applies BN post-scatter on the [16,1024] patch tile.
  - Out-of-image halo rows are zeroed with a per-core mask input (SPMD-safe).
  - Patches [16,1024] + (ones, sq) rows AllGathered; D tile = one f32r K=18
    matmul per [128,512] (4-way tile_position row packing), DVE relu (bf16),
    ACT sqrt (bf16). Symmetric-triangle: c0 rows compute all 16 col blocks,
    c1 rows only the c1 half; host mirrors the lower-left quadrant, diag=0.
"""
import numpy as np
import ml_dtypes
import concourse.bacc as bacc
import concourse.bass as bass
import concourse.tile as tile
from concourse import mybir
from concourse.bass_utils import run_bass_kernel_spmd

F32 = mybir.dt.float32
F32R = mybir.dt.float32r
BF16 = mybir.dt.bfloat16
AF = mybir.ActivationFunctionType
ALU = mybir.AluOpType

NCORES = 8
WP = 258            # padded row width (256 + 2 pad cols)
ROWS = 40           # ext slab rows per core (32 owned + 4 halo each side)
LEAD = 4            # lead margin so tap offsets never go negative
HROWS = 42          # slab rows + 1 pad row top/bottom
HFREE = LEAD + HROWS * WP + 4   # 10844
YFREE = ROWS * WP   # 10320
EPS = 1e-5
GOFF = [0, 64, 128, 192]        # g/be packing offsets per layer
COUT = [64, 64, 64, 2]

_CACHE = {}


def _conv_tiles(s0=0, s1=YFREE):
    out, s = [], s0
    while s < s1:
        L = min(512, s1 - s)
        out.append((s, L))
        s += L
    return out


def _c3_tiles():
    # conv3 tiles: moving [q0, q0+n), outputs [q0+1, q0+n-1)
    q_lo, q_hi = 4 * WP, 36 * WP
    out, q0 = [], q_lo - 1
    while q0 + 1 < q_hi:
        n = min(512, q_hi - q0 + 1)
        out.append((q0, n))
        q0 += 510
    return out


def build():
    nc = bacc.Bacc(trn_type="TRN2", num_devices=NCORES)
    x0 = nc.dram_tensor("x0", [27, YFREE], F32, kind="ExternalInput").ap()
    w0T = nc.dram_tensor("w0T", [27, 64], F32, kind="ExternalInput").ap()
    wp_in, ws_in = {}, {}
    for l in (1, 2):
        wp_in[l] = nc.dram_tensor(f"wp{l}", [3, 128, 64], F32, kind="ExternalInput").ap()
        ws_in[l] = nc.dram_tensor(f"ws{l}", [3, 64, 64], F32, kind="ExternalInput").ap()
    wA3_in = nc.dram_tensor("wA3", [128, 66], F32, kind="ExternalInput").ap()
    wB3_in = nc.dram_tensor("wB3", [64, 66], F32, kind="ExternalInput").ap()
    g_col = nc.dram_tensor("g_col", [64, 4], F32, kind="ExternalInput").ap()
    be_col = nc.dram_tensor("be_col", [64, 4], F32, kind="ExternalInput").ap()
    mask8 = nc.dram_tensor("mask8", [1, 8 * WP], F32, kind="ExternalInput").ap()
    out = nc.dram_tensor("out", [1024, 8192], BF16, kind="ExternalOutput").ap()

    TILES = {0: _conv_tiles(WP, 39 * WP),
             1: _conv_tiles(2 * WP, 38 * WP),
             2: _conv_tiles(3 * WP, 37 * WP)}

    with tile.TileContext(nc) as tc:
      with tc.tile_pool(name="pers", bufs=1) as pers, \
           tc.tile_pool(name="dr", bufs=1, space="DRAM") as dr:
        gsb = pers.tile([64, 4], F32)
        nc.sync.dma_start(out=gsb, in_=g_col)
        besb = pers.tile([64, 4], F32)
        nc.sync.dma_start(out=besb, in_=be_col)
        epsC = pers.tile([64, 1], F32)
        nc.vector.memset(epsC, EPS)

        def bn_finish(l, C, regions, bnps, sbp, transpose=True):
            """Cross-core BN: partial stats -> AllGather -> [C,1] scale/shift
            computed directly with free-dim reductions (no PE transposes)."""
            n = sum(r.shape[1] if r.ndim == 3 else 1 for r in regions)
            st = sbp.tile([C, n, 6], F32, tag=f"st{l}")
            i = 0
            for ap in regions:
                k = ap.shape[1] if ap.ndim == 3 else 1
                o = st[:, i:i + k, :] if ap.ndim == 3 else st[:, i, :]
                nc.vector.bn_stats(out=o, in_=ap)
                i += k
            mvt = sbp.tile([C, 2], F32, tag=f"mv{l}")
            nc.vector.bn_aggr(out=mvt, in_=st)
            sti = dr.tile([C, 2], F32, tag=f"sti{l}")
            sto = dr.tile([NCORES, C, 2], F32, tag=f"sto{l}")
            nc.gpsimd.dma_start(out=sti, in_=mvt)
            nc.gpsimd.collective_compute(
                "AllGather", ALU.bypass,
                replica_groups=[list(range(NCORES))],
                ins=[sti.opt()], outs=[sto.opt()])
            Gt = sbp.tile([C, 8, 2], F32, tag=f"Gt{l}")
            nc.sync.dma_start(out=Gt, in_=sto.rearrange("k c two -> c k two"))
            m, v = Gt[:, :, 0], Gt[:, :, 1]
            m2 = sbp.tile([C, 8], F32, tag=f"m2{l}")
            nc.vector.tensor_mul(m2, m, m)
            X = mybir.AxisListType.X
            S1 = sbp.tile([C, 1], F32, tag=f"S1{l}")
            nc.vector.tensor_reduce(S1, m, X, ALU.add)
            S2 = sbp.tile([C, 1], F32, tag=f"S2{l}")
            nc.vector.tensor_reduce(S2, v, X, ALU.add)
            S3 = sbp.tile([C, 1], F32, tag=f"S3{l}")
            nc.vector.tensor_reduce(S3, m2, X, ALU.add)
            am = sbp.tile([C, 1], F32, tag=f"am{l}")
            nc.vector.tensor_scalar_mul(am, S1, 0.125)
            t1 = sbp.tile([C, 1], F32, tag=f"t1{l}")
            nc.vector.tensor_mul(t1, am, am)        # E[m]^2
            t2 = sbp.tile([C, 1], F32, tag=f"t2{l}")
            nc.vector.tensor_add(t2, S2, S3)
            t3 = sbp.tile([C, 1], F32, tag=f"t3{l}")
            nc.vector.tensor_scalar_mul(t3, t2, 0.125)  # E[var] + E[m^2]
            t4 = sbp.tile([C, 1], F32, tag=f"t4{l}")
            nc.vector.tensor_sub(t4, t3, t1)        # total var
            sd = sbp.tile([C, 1], F32, tag=f"sd{l}")
            nc.scalar.activation(sd, t4, AF.Sqrt, bias=epsC[0:C])
            rs = sbp.tile([C, 1], F32, tag=f"rs{l}")
            nc.vector.reciprocal(rs, sd)
            scl = sbp.tile([C, 1], F32, tag=f"scl{l}")
            nc.vector.tensor_mul(scl, gsb[0:C, l:l + 1], rs)
            sh0 = sbp.tile([C, 1], F32, tag=f"sh0{l}")
            nc.vector.tensor_mul(sh0, am, scl)
            sh = sbp.tile([C, 1], F32, tag=f"sh{l}")
            nc.vector.tensor_sub(sh, besb[0:C, l:l + 1], sh0)
            return scl, sh

        # ---------------- conv phase ----------------
        with tc.tile_pool(name="cb", bufs=1) as cb, \
             tc.tile_pool(name="hp", bufs=1) as hp, \
             tc.tile_pool(name="c3p", bufs=4) as c3p, \
             tc.tile_pool(name="cps", bufs=6, space="PSUM") as cps, \
             tc.tile_pool(name="bnps", bufs=2, space="PSUM") as bnps:
            w0 = cb.tile([27, 64], F32R)
            nc.gpsimd.dma_start(out=w0, in_=w0T)
            x0t = cb.tile([27, YFREE], F32R)
            nc.gpsimd.dma_start(out=x0t, in_=x0)
            wpair, wsing = {}, {}
            for l in (1, 2):
                wpl = cb.tile([128, 192], F32R, tag=f"twp{l}")
                nc.gpsimd.dma_start(
                    out=wpl.rearrange("p (t c) -> p t c", t=3),
                    in_=wp_in[l].rearrange("t p c -> p t c"))
                wsl = cb.tile([64, 192], F32R, tag=f"tws{l}")
                nc.gpsimd.dma_start(
                    out=wsl.rearrange("p (t c) -> p t c", t=3),
                    in_=ws_in[l].rearrange("t p c -> p t c"))
                for p in range(3):
                    wpair[(l, p)] = wpl[:, p * 64:(p + 1) * 64]
                    wsing[(l, p)] = wsl[:, p * 64:(p + 1) * 64]
            mskf = cb.tile([64, 8 * WP], F32)
            nc.sync.dma_start(out=mskf, in_=mask8.partition_broadcast(64))
            mv_ = mskf.rearrange("p (r c) -> p r c", c=WP)
            wA3 = cb.tile([128, 66], F32R)
            nc.gpsimd.dma_start(out=wA3, in_=wA3_in)
            wB3 = cb.tile([64, 66], F32R)
            nc.gpsimd.dma_start(out=wB3, in_=wB3_in)

            def finish_layer(l, y):
                """BN + ReLU + mask + build padded f32r slab with shifted copy.

                Split into two column halves so the next conv's first tile
                group can start while the second half is still finishing.
                h margins are static zeros after layer 0 (hp bufs=1)."""
                yv = y.rearrange("p (r c) -> p r c", c=WP)
                regs = [yv[:, r, 1:257] for r in range(4, 36)]
                sbs, sbh = bn_finish(l, 64, regs, bnps, cb)
                h = hp.tile([128, HFREE], F32R, tag="h")
                M0 = 20 * WP
                hvA = h[0:64, LEAD + WP:LEAD + WP + YFREE]
                hv = hvA.rearrange("p (r c) -> p r c", c=WP)
                hcv = hvA.rearrange("p (r c) -> p c r", c=WP)
                if l == 0:
                    nc.vector.memset(h[0:64, 0:LEAD + WP].bitcast(F32), 0.0)
                    nc.vector.memset(
                        h[0:64, LEAD + WP + YFREE:HFREE].bitcast(F32), 0.0)
                    nc.vector.memset(
                        h[64:128, HFREE - WP:HFREE].bitcast(F32), 0.0)
                # first half (slab rows 0..20)
                nc.scalar.activation(hvA[:, 0:M0], y[:, 0:M0],
                                     AF.Relu, bias=sbh, scale=sbs)
                nc.vector.tensor_mul(hv[:, 0:4, :], hv[:, 0:4, :],
                                     mv_[:, 0:4, :])
                nc.vector.memset(hcv[:, 0, 0:20].bitcast(F32), 0.0)
                nc.vector.memset(hcv[:, 257, 0:20].bitcast(F32), 0.0)
                nc.vector.tensor_copy(h[64:128, 0:LEAD + M0],
                                      h[0:64, WP:LEAD + M0 + WP])
                # second half
                nc.scalar.activation(hvA[:, M0:YFREE], y[:, M0:YFREE],
                                     AF.Relu, bias=sbh, scale=sbs)
                nc.vector.tensor_mul(hv[:, 36:40, :], hv[:, 36:40, :],
                                     mv_[:, 4:8, :])
                nc.vector.memset(hcv[:, 0, 20:40].bitcast(F32), 0.0)
                nc.vector.memset(hcv[:, 257, 20:40].bitcast(F32), 0.0)
                nc.vector.tensor_copy(h[64:128, LEAD + M0:HFREE - WP],
                                      h[0:64, LEAD + M0 + WP:HFREE])
                return h

            # conv0 (im2col input, K=27, one stream)
            if True:
                y = cb.tile([64, YFREE], F32, tag="y")
                for (s, L) in TILES[0]:
                    ps = cps.tile([64, 512], F32, tag="cps")
                    nc.tensor.matmul(ps[:, 0:L], w0, x0t[:, s:s + L],
                                     start=True, stop=True)
                    nc.scalar.copy(y[:, s:s + L], ps[:, 0:L])
                h = finish_layer(0, y)

            # conv1, conv2 (6 streams: 3 pairs K=128 + 3 singles K=64)
            GROUP = 6
            for l in (1, 2):
                y = cb.tile([64, YFREE], F32, tag="y")
                for g0 in range(0, len(TILES[l]), GROUP):
                    grp = TILES[l][g0:g0 + GROUP]
                    pss = [cps.tile([64, 512], F32, tag="cps", name=f"cps{g0}_{i}")
                           for i in range(len(grp))]
                    for p in range(3):
                        for ps, (s, L) in zip(pss, grp):
                            o = LEAD + 516 + s + p - 1
                            nc.tensor.matmul(ps[:, 0:L],
                                             wsing[(l, p)],
                                             h[0:64, o:o + L],
                                             start=(p == 0), stop=False)
                    for p in range(3):
                        for ps, (s, L) in zip(pss, grp):
                            o = LEAD + s + p - 1
                            nc.tensor.matmul(ps[:, 0:L],
                                             wpair[(l, p)],
                                             h[0:128, o:o + L],
                                             start=False, stop=(p == 2))
                    for ps, (s, L) in zip(pss, grp):
                        nc.scalar.copy(y[:, s:s + L], ps[:, 0:L])
                h = finish_layer(l, y)

            # conv3: wide-stationary, patch-order moving. Stream A (K=128):
            # rows (r-1, r), 6 stationary cols (3 tap-cols x 2 ch); stream B
            # (K=64): row r+1, accumulated into the same [6,512] psum. DVE
            # slice-adds combine the 3 tap columns (px/gx shifts, edges via
            # slice bounds = implicit zero pad). Output lands patch-major.
            def c3mov(p0, np_, off):
                wide = h[p0:p0 + np_, off:off + 2 * WP]
                w2 = wide.rearrange("p (py c) -> p py c", py=2)[:, :, 0:256]
                return w2.rearrange("p py (gx px) -> p py px gx", px=4)

            y3rf = cb.tile([64, YFREE], F32, tag="y")
            y3 = y3rf[0:2, 0:8192]
            for gy in range(8):
                for ph in range(2):
                    offA = LEAD + (4 + 4 * gy + 2 * ph) * WP + 1
                    ps3 = cps.tile([66, 512], F32, tag="cps",
                                   name=f"c3_{gy}_{ph}")
                    nc.tensor.matmul(ps3, wA3,
                                     c3mov(0, 128, offA),
                                     start=True, stop=False)
                    nc.tensor.matmul(ps3, wB3,
                                     c3mov(0, 64, offA + 2 * WP),
                                     start=False, stop=True)
                    t0 = (gy * 2 + ph) * 512
                    yt = y3[:, t0:t0 + 512].rearrange(
                        "p (py px gx) -> p py px gx", py=2, px=4)
                    nc.scalar.copy(y3[:, t0:t0 + 512], ps3[32:34, :])
                    p0 = ps3[0:2, :].rearrange("q (py px gx) -> q py px gx",
                                               py=2, px=4)
                    p2 = ps3[64:66, :].rearrange("q (py px gx) -> q py px gx",
                                                 py=2, px=4)
                    nc.vector.tensor_add(yt[:, :, 1:4, :], yt[:, :, 1:4, :],
                                         p0[:, :, 0:3, :])
                    nc.vector.tensor_add(yt[:, :, 0, 1:], yt[:, :, 0, 1:],
                                         p0[:, :, 3, 0:63])
                    nc.vector.tensor_add(yt[:, :, 0:3, :], yt[:, :, 0:3, :],
                                         p2[:, :, 1:4, :])
                    nc.vector.tensor_add(yt[:, :, 3, 0:63], yt[:, :, 3, 0:63],
                                         p2[:, :, 0, 1:64])
            scl3, sh3 = bn_finish(3, 2,
                                  [y3[:, i * 512:(i + 1) * 512]
                                   for i in range(16)], bnps, cb,
                                  transpose=False)

            # ---------------- patch build + AllGather ----------------
            # scatter raw conv3 rows -> patch-major [16,1024], then apply BN3
            # (relu(scale*x+shift)) per channel half with broadcast scale.
            agin = dr.tile([18, 1024], F32, tag="agin")
            gath = dr.tile([8, 18, 1024], F32, tag="gath")
            scb = dr.tile([1, 4], F32, tag="scb")
            nc.sync.dma_start(out=scb[:, 0:2], in_=scl3)
            nc.sync.dma_start(out=scb[:, 2:4], in_=sh3)
            ssb = cb.tile([16, 4], F32)
            nc.sync.dma_start(out=ssb, in_=scb.partition_broadcast(16))
            y3d = dr.tile([16, 1024], F32, tag="y3d")
            y3dr = y3d.rearrange("k (c gy gx) -> k c gy gx", c=2, gy=8)
            y5 = y3.rearrange("p (gy ph py px gx) -> p gy ph py px gx",
                              gy=8, ph=2, py=2, px=4)
            for PY in range(4):
                for px in range(4):
                    nc.sync.dma_start(out=y3dr[PY * 4 + px],
                                      in_=y5[:, :, PY // 2, PY % 2, px, :])
            y3p = cb.tile([16, 1024], F32)
            nc.sync.dma_start(out=y3p, in_=y3d)
            for c in range(2):
                nc.scalar.activation(y3p[:, c * 512:(c + 1) * 512],
                                     y3p[:, c * 512:(c + 1) * 512],
                                     AF.Relu, bias=ssb[:, 2 + c:3 + c],
                                     scale=ssb[:, c:c + 1])
            Q = cb.tile([16, 1024], F32)
            nc.vector.tensor_mul(Q, y3p, y3p)
            ones16 = cb.tile([16, 1], F32)
            nc.vector.memset(ones16, 1.0)
            sqv = cb.tile([1, 1024], F32)
            for j in range(2):
                pq = bnps.tile([1, 512], F32, tag="bn")
                nc.tensor.matmul(pq, ones16, Q[:, j * 512:(j + 1) * 512],
                                 start=True, stop=True)
                nc.scalar.copy(sqv[:, j * 512:(j + 1) * 512], pq)
            ones1k = cb.tile([1, 1024], F32)
            nc.vector.memset(ones1k, 1.0)
            nc.sync.dma_start(out=agin[0:16, :], in_=y3p)
            nc.sync.dma_start(out=agin[16:17, :], in_=ones1k)
            nc.sync.dma_start(out=agin[17:18, :], in_=sqv)
            nc.gpsimd.collective_compute(
                "AllGather", ALU.bypass,
                replica_groups=[list(range(NCORES))],
                ins=[agin.opt()], outs=[gath.opt()])

        # ---------------- distance phase ----------------
        # Symmetric-triangle: c0 rows (m 0-3) compute all 16 col blocks; c1
        # rows (m 4-7) only the c1 half (n 8-15); host mirrors the lower-left
        # cross quadrant. agin rows are [p, 1, sq]; lhsT = [-2p, sq, 1] so
        # D^2 = -2 p_i.p_j + sq_i + sq_j in one K=18 f32r matmul.
        with tc.tile_pool(name="dist", bufs=1) as dist, \
             tc.tile_pool(name="stg", bufs=2) as stg, \
             tc.tile_pool(name="dps", bufs=8, space="PSUM") as dps:
            lhsT = dist.tile([128, 1024], F32R)
            nc.gpsimd.dma_start(out=lhsT[0:16, :], in_=agin[0:16, :])
            nc.gpsimd.dma_start(out=lhsT[16:17, :], in_=agin[17:18, :])
            nc.gpsimd.dma_start(out=lhsT[17:18, :], in_=agin[16:17, :])
            nc.vector.tensor_scalar_mul(lhsT[0:16, :], lhsT[0:16, :], -2.0)
            rhs = dist.tile([128, 8192], F32R)
            for c in range(2):
                nc.gpsimd.dma_start(
                    out=rhs[0:18, c * 4096:(c + 1) * 4096].rearrange(
                        "a (k n) -> a k n", k=8),
                    in_=gath[:, :, c * 512:(c + 1) * 512].rearrange(
                        "k a n -> a k n"))
            # replicate the 18 aug rows into 4 PE row-group strips so 4
            # K=18 matmuls run concurrently (tile_position row packing)
            for b in (32, 64, 96):
                nc.sync.dma_start(out=lhsT[b:b + 18, :], in_=lhsT[0:18, :])
                nc.sync.dma_start(out=rhs[b:b + 18, :], in_=rhs[0:18, :])
            ti = 0
            for m in range(8):
                nlist = range(16) if m < 4 else range(8, 16)
                stage = stg.tile([128, 8192], BF16, tag="stage")
                for n in nlist:
                    b = 32 * (ti % 4)
                    ti += 1
                    sl = stage[:, n * 512:(n + 1) * 512]
                    ps = dps.tile([128, 512], F32, tag="dp")
                    nc.tensor.matmul(ps,
                                     lhsT[b:b + 18, m * 128:(m + 1) * 128],
                                     rhs[b:b + 18, n * 512:(n + 1) * 512],
                                     start=True, stop=True,
                                     tile_position=(b, 0))
                    nc.vector.tensor_scalar_max(sl, ps, 0.0)
                    nc.scalar.activation(sl, sl, AF.Sqrt)
                c0 = 0 if m < 4 else 4096
                nc.sync.dma_start(out=out[m * 128:(m + 1) * 128, c0:8192],
                                  in_=stage[:, c0:8192])
    nc.finalize()
    return nc


def _prep_inputs(x, ws_, gs, bes):
    """Per-core numpy input dicts."""
    xp = np.pad(x[0], ((0, 0), (5, 5), (2, 3))).astype(np.float32)
    w0 = ws_[0]
    w0T = np.ascontiguousarray(
        w0.transpose(2, 3, 1, 0).reshape(27, 64)).astype(np.float32)
    wp, wsg = {}, {}
    for l in (1, 2):
        w = ws_[l]
        wp[l] = np.ascontiguousarray(np.stack(
            [np.concatenate([w[:, :, 0, p].T, w[:, :, 1, p].T], 0)
             for p in range(3)])).astype(np.float32)
        wsg[l] = np.ascontiguousarray(np.stack(
            [w[:, :, 2, p].T for p in range(3)])).astype(np.float32)
    w3 = ws_[3]
    wA3 = np.zeros((128, 66), np.float32)
    wB3 = np.zeros((64, 66), np.float32)
    for t in range(3):
        for ch in range(2):
            wA3[0:64, 32 * t + ch] = w3[ch, :, 0, t]
            wA3[64:128, 32 * t + ch] = w3[ch, :, 1, t]
            wB3[0:64, 32 * t + ch] = w3[ch, :, 2, t]
    g_col = np.zeros((64, 4), np.float32)
    be_col = np.zeros((64, 4), np.float32)
    for l in range(4):
        g_col[0:COUT[l], l] = np.asarray(gs[l], np.float32).ravel()
        be_col[0:COUT[l], l] = np.asarray(bes[l], np.float32).ravel()
    in_maps = []
    for k in range(NCORES):
        col = np.empty((27, ROWS, WP), np.float32)
        for dy in range(3):
            for dx in range(3):
                for ci in range(3):
                    r0 = 32 * k + dy
                    col[(dy * 3 + dx) * 3 + ci] = xp[ci, r0:r0 + ROWS, dx:dx + WP]
        mask = np.zeros((8, WP), np.float32)
        for i, r in enumerate([0, 1, 2, 3, 36, 37, 38, 39]):
            ir = 32 * k - 4 + r
            if 0 <= ir < 256:
                mask[i, 1:257] = 1.0
        in_maps.append(dict(
            x0=np.ascontiguousarray(col.reshape(27, YFREE)),
            w0T=w0T, wp1=wp[1], ws1=wsg[1], wp2=wp[2], ws2=wsg[2],
            wA3=wA3, wB3=wB3, g_col=g_col, be_col=be_col,
            mask8=np.ascontiguousarray(mask.reshape(1, 8 * WP))))
    return in_maps


def kernel(x, w0, b0, g0, be0, w1, b1, g1, be1, w2, b2, g2, be2,
           w3, b3, g3, be3):
    # conv bias b_i cancels exactly inside BatchNorm (mean absorbs it); unused.
    if "nc" not in _CACHE:
        _CACHE["nc"] = build()
    nc = _CACHE["nc"]
    in_maps = _prep_inputs(
        np.asarray(x, np.float32),
        [np.asarray(w, np.float32) for w in (w0, w1, w2, w3)],
        (g0, g1, g2, g3), (be0, be1, be2, be3))
    res = run_bass_kernel_spmd(nc, in_maps, list(range(NCORES)))
    D = np.empty((8192, 8192), np.float32)
    for k in range(NCORES):
        o = res.results[k]["out"]  # [1024, 8192] bf16
        D[k * 512:(k + 1) * 512, :] = o[0:512, :].astype(np.float32)
        D[4096 + k * 512: 4096 + (k + 1) * 512, 4096:] = \
            o[512:1024, 4096:].astype(np.float32)
    D[4096:, :4096] = D[:4096, 4096:].T
    np.fill_diagonal(D, 0.0)   # exact: d(i,i)=0; device f32r rounding leaves ~0.1
    return D


# revision 28
# speedup vs baseline: 1.3095x; 1.0791x over previous
"""TRN2 Bass kernel for nn_KNN_model (conv stack + pairwise patch distances).

Strategy (8 NeuronCores, SPMD):
  - Convs sharded over H: each core computes a 40-row slab (32 owned + 4 halo
    each side) through all 4 conv+BN+ReLU layers in float32r (TF32-like) on PE.
    3x3 conv = 6 matmul streams per tile: 3 K=128 pairs (top+mid tap rows via a
    partition-shifted slab copy) + 3 K=64 singles (bottom tap row).
  - conv3 (2 out ch): wide-stationary form - one K=128 stream (top+mid rows,
    3 tap-cols x 2 ch = 6 psum rows) + one K=64 stream (bottom row) into the
    same [6,512] psum; DVE shift-adds combine the 3 tap columns.
  - BN stats: per-core partial (mean, var) via one 3D bn_stats/bn_aggr over
    owned rows, tiny AllGather + PE ones-matmul reduce; layers 0-2 transpose
    scale/shift to [C,1] via K=1 matmuls and apply fused in one ACT pass;
    layer 3 applies BN post-scatter on the [16,1024] patch tile.
  - Out-of-image halo rows are zeroed with a per-core mask input (SPMD-safe).
  - Patches [16,1024] + (ones, sq) rows AllGathered; D tile = one f32r K=18
    matmul per [128,512] (4-way tile_position row packing), DVE relu (bf16),
    ACT sqrt (bf16). Symmetric-triangle: c0 rows compute all 16 col blocks,
    c1 rows only the c1 half; host mirrors the lower-left quadrant, diag=0.
"""
import numpy as np
import ml_dtypes
import concourse.bacc as bacc
import concourse.bass as bass
import concourse.tile as tile
from concourse import mybir
from concourse.bass_utils import run_bass_kernel_spmd

F32 = mybir.dt.float32
F32R = mybir.dt.float32r
BF16 = mybir.dt.bfloat16
AF = mybir.ActivationFunctionType
ALU = mybir.AluOpType

NCORES = 8
WP = 258            # padded row width (256 + 2 pad cols)
ROWS = 40           # ext slab rows per core (32 owned + 4 halo each side)
LEAD = 4            # lead margin so tap offsets never go negative
HROWS = 42          # slab rows + 1 pad row top/bottom
HFREE = LEAD + HROWS * WP + 4   # 10844
YFREE = ROWS * WP   # 10320
EPS = 1e-5
GOFF = [0, 64, 128, 192]        # g/be packing offsets per layer
COUT = [64, 64, 64, 2]

_CACHE = {}


def _conv_tiles(s0=0, s1=YFREE):
    out, s = [], s0
    while s < s1:
        L = min(512, s1 - s)
        out.append((s, L))
        s += L
    return out


def _c3_tiles():
    # conv3 tiles: moving [q0, q0+n), outputs [q0+1, q0+n-1)
    q_lo, q_hi = 4 * WP, 36 * WP
    out, q0 = [], q_lo - 1
    while q0 + 1 < q_hi:
        n = min(512, q_hi - q0 + 1)
        out.append((q0, n))
        q0 += 510
    return out


def build():
    nc = bacc.Bacc(trn_type="TRN2", num_devices=NCORES)
    x0 = nc.dram_tensor("x0", [27, YFREE], F32, kind="ExternalInput").ap()
    w0T = nc.dram_tensor("w0T", [27, 64], F32, kind="ExternalInput").ap()
    wp_in, ws_in = {}, {}
    for l in (1, 2):
        wp_in[l] = nc.dram_tensor(f"wp{l}", [3, 128, 64], F32, kind="ExternalInput").ap()
        ws_in[l] = nc.dram_tensor(f"ws{l}", [3, 64, 64], F32, kind="ExternalInput").ap()
    wA3_in = nc.dram_tensor("wA3", [128, 66], F32, kind="ExternalInput").ap()
    wB3_in = nc.dram_tensor("wB3", [64, 66], F32, kind="ExternalInput").ap()
    g_col = nc.dram_tensor("g_col", [64, 4], F32, kind="ExternalInput").ap()
    be_col = nc.dram_tensor("be_col", [64, 4], F32, kind="ExternalInput").ap()
    mask8 = nc.dram_tensor("mask8", [1, 8 * WP], F32, kind="ExternalInput").ap()
    out = nc.dram_tensor("out", [1024, 8192], BF16, kind="ExternalOutput").ap()

    TILES = {0: _conv_tiles(WP, 39 * WP),
             1: _conv_tiles(2 * WP, 38 * WP),
             2: _conv_tiles(3 * WP, 37 * WP)}

    with tile.TileContext(nc) as tc:
      with tc.tile_pool(name="pers", bufs=1) as pers, \
           tc.tile_pool(name="dr", bufs=1, space="DRAM") as dr:
        gsb = pers.tile([64, 4], F32)
        nc.sync.dma_start(out=gsb, in_=g_col)
        besb = pers.tile([64, 4], F32)
        nc.sync.dma_start(out=besb, in_=be_col)
        epsC = pers.tile([64, 1], F32)
        nc.vector.memset(epsC, EPS)

        def bn_finish(l, C, regions, bnps, sbp, transpose=True):
            """Cross-core BN: partial stats -> AllGather -> [C,1] scale/shift
            computed directly with free-dim reductions (no PE transposes)."""
            n = sum(r.shape[1] if r.ndim == 3 else 1 for r in regions)
            st = sbp.tile([C, n, 6], F32, tag=f"st{l}")
            i = 0
            for ap in regions:
                k = ap.shape[1] if ap.ndim == 3 else 1
                o = st[:, i:i + k, :] if ap.ndim == 3 else st[:, i, :]
                nc.vector.bn_stats(out=o, in_=ap)
                i += k
            mvt = sbp.tile([C, 2], F32, tag=f"mv{l}")
            nc.vector.bn_aggr(out=mvt, in_=st)
            sti = dr.tile([C, 2], F32, tag=f"sti{l}")
            sto = dr.tile([NCORES, C, 2], F32, tag=f"sto{l}")
            nc.gpsimd.dma_start(out=sti, in_=mvt)
            nc.gpsimd.collective_compute(
                "AllGather", ALU.bypass,
                replica_groups=[list(range(NCORES))],
                ins=[sti.opt()], outs=[sto.opt()])
            Gt = sbp.tile([C, 8, 2], F32, tag=f"Gt{l}")
            nc.sync.dma_start(out=Gt, in_=sto.rearrange("k c two -> c k two"))
            m, v = Gt[:, :, 0], Gt[:, :, 1]
            m2 = sbp.tile([C, 8], F32, tag=f"m2{l}")
            nc.vector.tensor_mul(m2, m, m)
            X = mybir.AxisListType.X
            S1 = sbp.tile([C, 1], F32, tag=f"S1{l}")
            nc.vector.tensor_reduce(S1, m, X, ALU.add)
            S2 = sbp.tile([C, 1], F32, tag=f"S2{l}")
            nc.vector.tensor_reduce(S2, v, X, ALU.add)
            S3 = sbp.tile([C, 1], F32, tag=f"S3{l}")
            nc.vector.tensor_reduce(S3, m2, X, ALU.add)
            am = sbp.tile([C, 1], F32, tag=f"am{l}")
            nc.vector.tensor_scalar_mul(am, S1, 0.125)
            t1 = sbp.tile([C, 1], F32, tag=f"t1{l}")
            nc.vector.tensor_mul(t1, am, am)        # E[m]^2
            t2 = sbp.tile([C, 1], F32, tag=f"t2{l}")
            nc.vector.tensor_add(t2, S2, S3)
            t3 = sbp.tile([C, 1], F32, tag=f"t3{l}")
            nc.vector.tensor_scalar_mul(t3, t2, 0.125)  # E[var] + E[m^2]
            t4 = sbp.tile([C, 1], F32, tag=f"t4{l}")
            nc.vector.tensor_sub(t4, t3, t1)        # total var
            sd = sbp.tile([C, 1], F32, tag=f"sd{l}")
            nc.scalar.activation(sd, t4, AF.Sqrt, bias=epsC[0:C])
            rs = sbp.tile([C, 1], F32, tag=f"rs{l}")
            nc.vector.reciprocal(rs, sd)
            scl = sbp.tile([C, 1], F32, tag=f"scl{l}")
            nc.vector.tensor_mul(scl, gsb[0:C, l:l + 1], rs)
            sh0 = sbp.tile([C, 1], F32, tag=f"sh0{l}")
            nc.vector.tensor_mul(sh0, am, scl)
            sh = sbp.tile([C, 1], F32, tag=f"sh{l}")
            nc.vector.tensor_sub(sh, besb[0:C, l:l + 1], sh0)
            return scl, sh

        # ---------------- conv phase ----------------
        with tc.tile_pool(name="cb", bufs=1) as cb, \
             tc.tile_pool(name="hp", bufs=1) as hp, \
             tc.tile_pool(name="c3p", bufs=4) as c3p, \
             tc.tile_pool(name="cps", bufs=6, space="PSUM") as cps, \
             tc.tile_pool(name="bnps", bufs=2, space="PSUM") as bnps:
            w0 = cb.tile([27, 64], F32R)
            nc.gpsimd.dma_start(out=w0, in_=w0T)
            x0t = cb.tile([27, YFREE], F32R)
            nc.gpsimd.dma_start(out=x0t, in_=x0)
            wpair, wsing = {}, {}
            for l in (1, 2):
                wpl = cb.tile([128, 192], F32R, tag=f"twp{l}")
                nc.gpsimd.dma_start(
                    out=wpl.rearrange("p (t c) -> p t c", t=3),
                    in_=wp_in[l].rearrange("t p c -> p t c"))
                wsl = cb.tile([64, 192], F32R, tag=f"tws{l}")
                nc.gpsimd.dma_start(
                    out=wsl.rearrange("p (t c) -> p t c", t=3),
                    in_=ws_in[l].rearrange("t p c -> p t c"))
                for p in range(3):
                    wpair[(l, p)] = wpl[:, p * 64:(p + 1) * 64]
                    wsing[(l, p)] = wsl[:, p * 64:(p + 1) * 64]
            mskf = cb.tile([64, 8 * WP], F32)
            nc.sync.dma_start(out=mskf, in_=mask8.partition_broadcast(64))
            mv_ = mskf.rearrange("p (r c) -> p r c", c=WP)
            wA3 = cb.tile([128, 66], F32R)
            nc.gpsimd.dma_start(out=wA3, in_=wA3_in)
            wB3 = cb.tile([64, 66], F32R)
            nc.gpsimd.dma_start(out=wB3, in_=wB3_in)

            def finish_layer(l, y):
                """BN + ReLU + mask + build padded f32r slab with shifted copy.

                Split into two column halves so the next conv's first tile
                group can start while the second half is still finishing.
                h margins are static zeros after layer 0 (hp bufs=1)."""
                yv = y.rearrange("p (r c) -> p r c", c=WP)
                regs = [yv[:, r, 1:257] for r in range(4, 36)]
                sbs, sbh = bn_finish(l, 64, regs, bnps, cb)
                h = hp.tile([128, HFREE], F32R, tag="h")
                M0 = 20 * WP
                hvA = h[0:64, LEAD + WP:LEAD + WP + YFREE]
                hv = hvA.rearrange("p (r c) -> p r c", c=WP)
                hcv = hvA.rearrange("p (r c) -> p c r", c=WP)
                if l == 0:
                    nc.vector.memset(h[0:64, 0:LEAD + WP].bitcast(F32), 0.0)
                    nc.vector.memset(
                        h[0:64, LEAD + WP + YFREE:HFREE].bitcast(F32), 0.0)
                    nc.vector.memset(
                        h[64:128, HFREE - WP:HFREE].bitcast(F32), 0.0)
                # first half (slab rows 0..20)
                nc.scalar.activation(hvA[:, 0:M0], y[:, 0:M0],
                                     AF.Relu, bias=sbh, scale=sbs)
                nc.vector.tensor_mul(hv[:, 0:4, :], hv[:, 0:4, :],
                                     mv_[:, 0:4, :])
                nc.vector.memset(hcv[:, 0, 0:20].bitcast(F32), 0.0)
                nc.vector.memset(hcv[:, 257, 0:20].bitcast(F32), 0.0)
                nc.vector.tensor_copy(h[64:128, 0:LEAD + M0],
                                      h[0:64, WP:LEAD + M0 + WP])
                # second half
                nc.scalar.activation(hvA[:, M0:YFREE], y[:, M0:YFREE],
                                     AF.Relu, bias=sbh, scale=sbs)
                nc.vector.tensor_mul(hv[:, 36:40, :], hv[:, 36:40, :],
                                     mv_[:, 4:8, :])
                nc.vector.memset(hcv[:, 0, 20:40].bitcast(F32), 0.0)
                nc.vector.memset(hcv[:, 257, 20:40].bitcast(F32), 0.0)
                nc.vector.tensor_copy(h[64:128, LEAD + M0:HFREE - WP],
                                      h[0:64, LEAD + M0 + WP:HFREE])
                return h

            # conv0 (im2col input, K=27, one stream)
            if True:
                y = cb.tile([64, YFREE], F32, tag="y")
                for (s, L) in TILES[0]:
                    ps = cps.tile([64, 512], F32, tag="cps")
                    nc.tensor.matmul(ps[:, 0:L], w0, x0t[:, s:s + L],
                                     start=True, stop=True)
                    nc.scalar.copy(y[:, s:s + L], ps[:, 0:L])
                h = finish_layer(0, y)

            # conv1, conv2 (6 streams: 3 pairs K=128 + 3 singles K=64)
            GROUP = 6
            for l in (1, 2):
                y = cb.tile([64, YFREE], F32, tag="y")
                for g0 in range(0, len(TILES[l]), GROUP):
                    grp = TILES[l][g0:g0 + GROUP]
                    pss = [cps.tile([64, 512], F32, tag="cps", name=f"cps{g0}_{i}")
                           for i in range(len(grp))]
                    for p in range(3):
                        for ps, (s, L) in zip(pss, grp):
                            o = LEAD + 516 + s + p - 1
                            nc.tensor.matmul(ps[:, 0:L],
                                             wsing[(l, p)],
                                             h[0:64, o:o + L],
                                             start=(p == 0), stop=False)
                    for p in range(3):
                        for ps, (s, L) in zip(pss, grp):
                            o = LEAD + s + p - 1
                            nc.tensor.matmul(ps[:, 0:L],
                                             wpair[(l, p)],
                                             h[0:128, o:o + L],
                                             start=False, stop=(p == 2))
                    for ps, (s, L) in zip(pss, grp):
                        nc.scalar.copy(y[:, s:s + L], ps[:, 0:L])
                h = finish_layer(l, y)

            # conv3: wide-stationary, patch-order moving. Stream A (K=128):
            # rows (r-1, r), 6 stationary cols (3 tap-cols x 2 ch); stream B
            # (K=64): row r+1, accumulated into the same [6,512] psum. DVE
            # slice-adds combine the 3 tap columns (px/gx shifts, edges via
            # slice bounds = implicit zero pad). Output lands patch-major.
            def c3mov(p0, np_, off):
                wide = h[p0:p0 + np_, off:off + 2 * WP]
                w2 = wide.rearrange("p (py c) -> p py c", py=2)[:, :, 0:256]
                return w2.rearrange("p py (gx px) -> p py px gx", px=4)

            y3rf = cb.tile([64, YFREE], F32, tag="y")
            y3 = y3rf[0:2, 0:8192]
            for gy in range(8):
                for ph in range(2):
                    offA = LEAD + (4 + 4 * gy + 2 * ph) * WP + 1
                    ps3 = cps.tile([66, 512], F32, tag="cps",
                                   name=f"c3_{gy}_{ph}")
                    nc.tensor.matmul(ps3, wA3,
                                     c3mov(0, 128, offA),
                                     start=True, stop=False)
                    nc.tensor.matmul(ps3, wB3,
                                     c3mov(0, 64, offA + 2 * WP),
                                     start=False, stop=True)
                    t0 = (gy * 2 + ph) * 512
                    yt = y3[:, t0:t0 + 512].rearrange(
                        "p (py px gx) -> p py px gx", py=2, px=4)
                    nc.scalar.copy(y3[:, t0:t0 + 512], ps3[32:34, :])
                    p0 = ps3[0:2, :].rearrange("q (py px gx) -> q py px gx",
                                               py=2, px=4)
                    p2 = ps3[64:66, :].rearrange("q (py px gx) -> q py px gx",
                                                 py=2, px=4)
                    nc.vector.tensor_add(yt[:, :, 1:4, :], yt[:, :, 1:4, :],
                                         p0[:, :, 0:3, :])
                    nc.vector.tensor_add(yt[:, :, 0, 1:], yt[:, :, 0, 1:],
                                         p0[:, :, 3, 0:63])
                    nc.vector.tensor_add(yt[:, :, 0:3, :], yt[:, :, 0:3, :],
                                         p2[:, :, 1:4, :])
                    nc.vector.tensor_add(yt[:, :, 3, 0:63], yt[:, :, 3, 0:63],
                                         p2[:, :, 0, 1:64])
            scl3, sh3 = bn_finish(3, 2,
                                  [y3[:, i * 512:(i + 1) * 512]
                                   for i in range(16)], bnps, cb,
                                  transpose=False)

            # ---------------- patch build + AllGather ----------------
            # scatter raw conv3 rows -> patch-major [16,1024], then apply BN3
            # (relu(scale*x+shift)) per channel half with broadcast scale.
            agin = dr.tile([18, 1024], F32, tag="agin")
            gath = dr.tile([8, 18, 1024], F32, tag="gath")
            scb = dr.tile([1, 4], F32, tag="scb")
            nc.sync.dma_start(out=scb[:, 0:2], in_=scl3)
            nc.sync.dma_start(out=scb[:, 2:4], in_=sh3)
            ssb = cb.tile([16, 4], F32)
            nc.sync.dma_start(out=ssb, in_=scb.partition_broadcast(16))
            y3d = dr.tile([16, 1024], F32, tag="y3d")
            y3dr = y3d.rearrange("k (c gy gx) -> k c gy gx", c=2, gy=8)
            y5 = y3.rearrange("p (gy ph py px gx) -> p gy ph py px gx",
                              gy=8, ph=2, py=2, px=4)
            for PY in range(4):
                for px in range(4):
                    nc.sync.dma_start(out=y3dr[PY * 4 + px],
                                      in_=y5[:, :, PY // 2, PY % 2, px, :])
            y3p = cb.tile([16, 1024], F32)
            nc.sync.dma_start(out=y3p, in_=y3d)
            for c in range(2):
                nc.scalar.activation(y3p[:, c * 512:(c + 1) * 512],
                                     y3p[:, c * 512:(c + 1) * 512],
                                     AF.Relu, bias=ssb[:, 2 + c:3 + c],
                                     scale=ssb[:, c:c + 1])
            Q = cb.tile([16, 1024], F32)
            nc.vector.tensor_mul(Q, y3p, y3p)
            ones16 = cb.tile([16, 1], F32)
            nc.vector.memset(ones16, 1.0)
            sqv = cb.tile([1, 1024], F32)
            for j in range(2):
                pq = bnps.tile([1, 512], F32, tag="bn")
                nc.tensor.matmul(pq, ones16, Q[:, j * 512:(j + 1) * 512],
                                 start=True, stop=True)
                nc.scalar.copy(sqv[:, j * 512:(j + 1) * 512], pq)
            ones1k = cb.tile([1, 1024], F32)
            nc.vector.memset(ones1k, 1.0)
            nc.sync.dma_start(out=agin[0:16, :], in_=y3p)
            nc.sync.dma_start(out=agin[16:17, :], in_=ones1k)
            nc.sync.dma_start(out=agin[17:18, :], in_=sqv)
            nc.gpsimd.collective_compute(
                "AllGather", ALU.bypass,
                replica_groups=[list(range(NCORES))],
                ins=[agin.opt()], outs=[gath.opt()])

        # ---------------- distance phase ----------------
        # Symmetric-triangle: c0 rows (m 0-3) compute all 16 col blocks; c1
        # rows (m 4-7) only the c1 half (n 8-15); host mirrors the lower-left
        # cross quadrant. agin rows are [p, 1, sq]; lhsT = [-2p, sq, 1] so
        # D^2 = -2 p_i.p_j + sq_i + sq_j in one K=18 f32r matmul.
        with tc.tile_pool(name="dist", bufs=1) as dist, \
             tc.tile_pool(name="stg", bufs=2) as stg, \
             tc.tile_pool(name="dps", bufs=8, space="PSUM") as dps:
            lhsT = dist.tile([18, 1024], F32R)
            nc.gpsimd.dma_start(out=lhsT[0:16, :], in_=agin[0:16, :])
            nc.gpsimd.dma_start(out=lhsT[16:17, :], in_=agin[17:18, :])
            nc.gpsimd.dma_start(out=lhsT[17:18, :], in_=agin[16:17, :])
            nc.vector.tensor_scalar_mul(lhsT[0:16, :], lhsT[0:16, :], -2.0)
            rhs = dist.tile([18, 8192], F32R)
            for c in range(2):
                nc.gpsimd.dma_start(
                    out=rhs[0:18, c * 4096:(c + 1) * 4096].rearrange(
                        "a (k n) -> a k n", k=8),
                    in_=gath[:, :, c * 512:(c + 1) * 512].rearrange(
                        "k a n -> a k n"))
            # single K=18 row strip: with f32r (1 cyc/row) the PE serial
            # time sits below the scalar-sqrt floor, so no row packing needed
            for m in range(8):
                nlist = range(16) if m < 4 else range(8, 16)
                stage = stg.tile([128, 8192], BF16, tag="stage")
                for n in nlist:
                    sl = stage[:, n * 512:(n + 1) * 512]
                    ps = dps.tile([128, 512], F32, tag="dp")
                    nc.tensor.matmul(ps,
                                     lhsT[0:18, m * 128:(m + 1) * 128],
                                     rhs[0:18, n * 512:(n + 1) * 512],
                                     start=True, stop=True)
                    nc.vector.tensor_scalar_max(sl, ps, 0.0)
                    nc.scalar.activation(sl, sl, AF.Sqrt)
                c0 = 0 if m < 4 else 4096
                nc.sync.dma_start(out=out[m * 128:(m + 1) * 128, c0:8192],
                                  in_=stage[:, c0:8192])
    nc.finalize()
    return nc


def _prep_inputs(x, ws_, gs, bes):
    """Per-core numpy input dicts."""
    xp = np.pad(x[0], ((0, 0), (5, 5), (2, 3))).astype(np.float32)
    w0 = ws_[0]
    w0T = np.ascontiguousarray(
        w0.transpose(2, 3, 1, 0).reshape(27, 64)).astype(np.float32)
    wp, wsg = {}, {}
    for l in (1, 2):
        w = ws_[l]
        wp[l] = np.ascontiguousarray(np.stack(
            [np.concatenate([w[:, :, 0, p].T, w[:, :, 1, p].T], 0)
             for p in range(3)])).astype(np.float32)
        wsg[l] = np.ascontiguousarray(np.stack(
            [w[:, :, 2, p].T for p in range(3)])).astype(np.float32)
    w3 = ws_[3]
    wA3 = np.zeros((128, 66), np.float32)
    wB3 = np.zeros((64, 66), np.float32)
    for t in range(3):
        for ch in range(2):
            wA3[0:64, 32 * t + ch] = w3[ch, :, 0, t]
            wA3[64:128, 32 * t + ch] = w3[ch, :, 1, t]
            wB3[0:64, 32 * t + ch] = w3[ch, :, 2, t]
    g_col = np.zeros((64, 4), np.float32)
    be_col = np.zeros((64, 4), np.float32)
    for l in range(4):
        g_col[0:COUT[l], l] = np.asarray(gs[l], np.float32).ravel()
        be_col[0:COUT[l], l] = np.asarray(bes[l], np.float32).ravel()
    in_maps = []
    for k in range(NCORES):
        col = np.empty((27, ROWS, WP), np.float32)
        for dy in range(3):
            for dx in range(3):
                for ci in range(3):
                    r0 = 32 * k + dy
                    col[(dy * 3 + dx) * 3 + ci] = xp[ci, r0:r0 + ROWS, dx:dx + WP]
        mask = np.zeros((8, WP), np.float32)
        for i, r in enumerate([0, 1, 2, 3, 36, 37, 38, 39]):
            ir = 32 * k - 4 + r
            if 0 <= ir < 256:
                mask[i, 1:257] = 1.0
        in_maps.append(dict(
            x0=np.ascontiguousarray(col.reshape(27, YFREE)),
            w0T=w0T, wp1=wp[1], ws1=wsg[1], wp2=wp[2], ws2=wsg[2],
            wA3=wA3, wB3=wB3, g_col=g_col, be_col=be_col,
            mask8=np.ascontiguousarray(mask.reshape(1, 8 * WP))))
    return in_maps


def kernel(x, w0, b0, g0, be0, w1, b1, g1, be1, w2, b2, g2, be2,
           w3, b3, g3, be3):
    # conv bias b_i cancels exactly inside BatchNorm (mean absorbs it); unused.
    if "nc" not in _CACHE:
        _CACHE["nc"] = build()
    nc = _CACHE["nc"]
    in_maps = _prep_inputs(
        np.asarray(x, np.float32),
        [np.asarray(w, np.float32) for w in (w0, w1, w2, w3)],
        (g0, g1, g2, g3), (be0, be1, be2, be3))
    res = run_bass_kernel_spmd(nc, in_maps, list(range(NCORES)))
    D = np.empty((8192, 8192), np.float32)
    for k in range(NCORES):
        o = res.results[k]["out"]  # [1024, 8192] bf16
        D[k * 512:(k + 1) * 512, :] = o[0:512, :].astype(np.float32)
        D[4096 + k * 512: 4096 + (k + 1) * 512, 4096:] = \
            o[512:1024, 4096:].astype(np.float32)
    D[4096:, :4096] = D[:4096, 4096:].T
    np.fill_diagonal(D, 0.0)   # exact: d(i,i)=0; device f32r rounding leaves ~0.1
    return D
